# revision 1
# baseline (speedup 1.0000x reference)
"""DeepSeek MLA prefill on 8 TRN2 NeuronCores.

Sharding: tensor-parallel over heads (2 heads/core) for the b-projections,
attention and w_o (row-parallel -> host sums partials); sequence-parallel
a-projections (each core computes 256 tokens of q_a/kv_a/k_pe, normalizes,
ropes k_pe, then on-device AllGathers replicate the 2112x256 activations).
The kv-group a-proj runs first so its (small) gather and the whole kv
b-projection overlap the q-group a-proj and gather.

All activations that feed matmuls are kept feature-major ([d, T]) so no
on-device transposes are needed; v is produced token-major directly.
Matmuls run in bf16 with f32 PSUM accumulation (rel-err gate is ~2e-2).
"""

import math
import os

import ml_dtypes
import numpy as np

import concourse.bacc as bacc
import concourse.mybir as mybir
import concourse.tile as tile
from concourse.bass_utils import run_bass_kernel_spmd

F32 = mybir.dt.float32
BF16 = mybir.dt.bfloat16
AF = mybir.ActivationFunctionType
ALU = mybir.AluOpType

# problem dims (hardcoded per contract)
T, HID, H = 2048, 5120, 16
QL, KL = 1536, 512
NOPE, ROPE, VD = 128, 64, 128
QK = NOPE + ROPE
EPS = 1e-6
NCORE = 8
HPC = H // NCORE          # heads per core = 2
TLOC = T // NCORE         # tokens per core = 256
P = 128
HCH = HID // P            # 40 hidden chunks
QLC = QL // P             # 12
KLC = KL // P             # 4
MT = QLC + KLC + 1        # 17 a-proj output tiles (12 q + 4 kv + 1 pe[64])
NKV = MT - QLC            # 5 kv-group tiles
NT = T // P               # 16 token tiles
NQS = 4                   # 512-wide q slices per head
NHS = HID // 512          # 10 output column slices

# yarn rope params
BASE, FACTOR = 10000.0, 40.0
BETA_FAST, BETA_SLOW, ORIG_MAX = 32.0, 1.0, 4096
MSCALE = 1.0
MSCALE_ALL_DIM = 1.0


def _yarn_get_mscale(scale, m):
    if scale <= 1.0:
        return 1.0
    return 0.1 * m * math.log(scale) + 1.0


def _yarn_inv_freq():
    pos_freqs = BASE ** (np.arange(0, ROPE, 2, dtype=np.float64) / ROPE)
    extra = 1.0 / pos_freqs
    inter = 1.0 / (FACTOR * pos_freqs)

    def corr_dim(n):
        return ROPE * math.log(ORIG_MAX / (n * 2 * math.pi)) / (2 * math.log(BASE))

    low = max(math.floor(corr_dim(BETA_FAST)), 0)
    high = min(math.ceil(corr_dim(BETA_SLOW)), ROPE - 1)
    ramp = np.clip(
        (np.arange(ROPE // 2, dtype=np.float64) - low) / max(high - low, 0.001),
        0.0,
        1.0,
    )
    mask = 1.0 - ramp
    return (inter * (1.0 - mask) + extra * mask).astype(np.float32)


COS_SIN_MSCALE = _yarn_get_mscale(FACTOR, MSCALE) / _yarn_get_mscale(
    FACTOR, MSCALE_ALL_DIM
)
_M = _yarn_get_mscale(FACTOR, MSCALE_ALL_DIM)
ATTN_SCALE = (QK ** -0.5) * _M * _M

BF = ml_dtypes.bfloat16
# de-interleave perm: even rope dims then odd rope dims
PE_PERM = np.concatenate([np.arange(0, ROPE, 2), np.arange(1, ROPE, 2)])

LAST_EXEC_NS = None


def _build_nc(single=False, reps=1):
    # single=True: no collective, 1 core — for cost-model timeline sims only
    nc = bacc.Bacc(
        "TRN2",
        target_bir_lowering=False,
        debug=False,
        num_devices=1 if single else NCORE,
    )

    hT = nc.dram_tensor("hT", [P, HCH, TLOC], BF16, kind="ExternalInput").ap()
    wa = nc.dram_tensor("wa", [MT, P, HCH, P], BF16, kind="ExternalInput").ap()
    wqb = nc.dram_tensor("wqb", [P, QLC, HPC * QK], BF16, kind="ExternalInput").ap()
    wkvb = nc.dram_tensor("wkvb", [P, KLC, 512], BF16, kind="ExternalInput").ap()
    wo = nc.dram_tensor("wo", [P, HPC, HID], BF16, kind="ExternalInput").ap()
    cosf = nc.dram_tensor("cosf", [ROPE // 2, T], BF16, kind="ExternalInput").ap()
    sinf = nc.dram_tensor("sinf", [ROPE // 2, T], BF16, kind="ExternalInput").ap()
    cosl = nc.dram_tensor("cosl", [ROPE, TLOC], BF16, kind="ExternalInput").ap()
    sinl = nc.dram_tensor("sinl", [ROPE, TLOC], BF16, kind="ExternalInput").ap()
    onesd = nc.dram_tensor("ones", [P, P], BF16, kind="ExternalInput").ap()
    trid = nc.dram_tensor("tri", [P, P], BF16, kind="ExternalInput").ap()
    out = nc.dram_tensor("out", [T, HID], BF16, kind="ExternalOutput").ap()

    locb = nc.dram_tensor("locb", [MT, P, TLOC], BF16).ap()
    gathkv = nc.dram_tensor(
        "gathkv", [NCORE, NKV, P, TLOC], BF16, addr_space="Shared"
    ).ap()
    gathq = nc.dram_tensor(
        "gathq", [NCORE, QLC, P, TLOC], BF16, addr_space="Shared"
    ).ap()

    with tile.TileContext(nc) as tc:
        with (
            tc.tile_pool(name="const", bufs=1) as cp,
            tc.tile_pool(name="persist", bufs=1) as pp,
            tc.tile_pool(name="ocp", bufs=4) as ocp,
        ):
            ones_sb = cp.tile([P, P], BF16, tag="ones")
            tri_sb = cp.tile([P, P], BF16, tag="tri")
            cosf_sb = cp.tile([ROPE // 2, T], BF16, tag="cosf")
            sinf_sb = cp.tile([ROPE // 2, T], BF16, tag="sinf")
            cosl_sb = cp.tile([ROPE, TLOC], BF16, tag="cosl")
            sinl_sb = cp.tile([ROPE, TLOC], BF16, tag="sinl")
            eps_sb = cp.tile([P, 1], F32, tag="eps")
            nc.vector.memset(eps_sb[:], EPS)

            # persistent attention operands (live across the phase transition)
            qTn = pp.tile([P, HPC, T], BF16, tag="qTn")
            # both heads' roped q_pe packed: rows [h0e h0o h1e h1o] x 32
            qTp = pp.tile([P, T], BF16, tag="qTp")
            kTn = pp.tile([P, HPC, T], BF16, tag="kTn")
            vtok = pp.tile([P, NT, HPC * VD], BF16, tag="vtok")
            OnT = pp.tile([P, HPC, T], BF16, tag="OnT")
            kag = pp.tile([P, KLC, NCORE, TLOC], BF16, tag="kag")
            # k_pe duplicated into both 64-row halves so each head's score
            # matmul has lhsT/rhs at the same base partition (0 or 64)
            kpe = pp.tile([P, NCORE, TLOC], BF16, tag="kpe")
            wkvb_sb = pp.tile([P, KLC, 512], BF16, tag="wkvb")
            wqb_sb = pp.tile([P, QLC, HPC * QK], BF16, tag="wqb")
            wo_sb = pp.tile([P, HPC, HID], BF16, tag="wo")
            kag_f = kag.rearrange("p m c t -> p m (c t)")

            for _rep in range(reps):
                # ---------------- phase 1: local a-projections ----------------
                with (
                    tc.tile_pool(name="p1", bufs=1) as p1,
                    tc.tile_pool(name="wap", bufs=3) as wap,
                    tc.tile_pool(name="sqp", bufs=3) as sqp,
                    tc.tile_pool(name="ps1", bufs=3, space="PSUM") as ps1,
                    tc.tile_pool(name="pss", bufs=1, space="PSUM") as pss,
                ):
                    hT_sb = p1.tile([P, HCH, TLOC], BF16, tag="hT")
                    # fine-split the head of the load so the first matmul
                    # starts early; coarse chunks for the rest
                    for k0, k1 in [(0, 2), (2, 4), (4, 13), (13, 22), (22, 31), (31, 40)]:
                        nc.sync.dma_start(
                            hT_sb[:, k0:k1, :], hT[:, k0:k1, :]
                        )
                    araw = p1.tile([P, MT, TLOC], BF16, tag="araw")
                    anrm = p1.tile([P, MT, TLOC], BF16, tag="anrm")
                    ssq = pss.tile([P, TLOC], F32, tag="ssq")
                    sskv = pss.tile([P, TLOC], F32, tag="sskv")

                    # kv-group mtiles first so their collective + the whole kv
                    # b-projection overlap the (3x bigger) q-group a-proj
                    for m in list(range(QLC, MT)) + list(range(QLC)):
                        wt = wap.tile([P, HCH, P], BF16, tag="wt")
                        if m == QLC:  # first mtile: fine-split so PE starts early
                            for kc in range(8):
                                nc.sync.dma_start(
                                    wt[:, kc * 5 : (kc + 1) * 5, :],
                                    wa[m, :, kc * 5 : (kc + 1) * 5, :],
                                )
                            # consts are small and first needed mid-phase-1:
                            # load them after the latency-critical first chunks
                            nc.sync.dma_start(ones_sb[:], onesd)
                            nc.sync.dma_start(cosl_sb[:], cosl)
                            nc.sync.dma_start(sinl_sb[:], sinl)
                            nc.sync.dma_start(tri_sb[:], trid)
                            nc.sync.dma_start(cosf_sb[:], cosf)
                            nc.sync.dma_start(sinf_sb[:], sinf)
                        else:
                            nc.sync.dma_start(wt[:], wa[m])
                        ps = ps1.tile([P, TLOC], F32, tag="aps")
                        for k in range(HCH):
                            nc.tensor.matmul(
                                ps[:],
                                wt[:, k, :],
                                hT_sb[:, k, :],
                                start=(k == 0),
                                stop=(k == HCH - 1),
                            )
                        nc.scalar.copy(araw[:, m, :], ps[:])
                        if m < QLC + KLC:
                            sq = sqp.tile([P, TLOC], BF16, tag="sq")
                            nc.scalar.activation(sq[:], ps[:], AF.Square)
                            if m < QLC:
                                nc.tensor.matmul(
                                    ssq[:],
                                    ones_sb[:],
                                    sq[:],
                                    start=(m == 0),
                                    stop=(m == QLC - 1),
                                    skip_group_check=True,
                                )
                            else:
                                nc.tensor.matmul(
                                    sskv[:],
                                    ones_sb[:],
                                    sq[:],
                                    start=(m == QLC),
                                    stop=(m == QLC + KLC - 1),
                                    skip_group_check=True,
                                )

                        if m == MT - 1:
                            # kv group locally complete: normalize, rope, ship
                            rsq_k = p1.tile([P, TLOC], F32, tag="rsq_k")
                            tmpf2 = p1.tile([P, TLOC], F32, tag="tmpf2")
                            nc.scalar.activation(
                                tmpf2[:], sskv[:], AF.Sqrt,
                                bias=eps_sb[:], scale=1.0 / KL,
                            )
                            nc.vector.reciprocal(rsq_k[:], tmpf2[:])
                            for mm in range(QLC, QLC + KLC):
                                nc.vector.tensor_mul(
                                    anrm[:, mm, :], araw[:, mm, :], rsq_k[:]
                                )
                            # rope k_pe (rows 0:32 even, 32:64 odd of tile MT-1).
                            # Two-SBUF-input ops must share base partition, so
                            # cos/sin tables are duplicated across both halves.
                            t1 = p1.tile([ROPE, TLOC], BF16, tag="t1")
                            t2 = p1.tile([ROPE, TLOC], BF16, tag="t2")
                            xe = araw[0:32, MT - 1, :]
                            xo = araw[32:64, MT - 1, :]
                            nc.vector.tensor_mul(t1[0:32, :], xe, cosl_sb[0:32, :])
                            nc.vector.tensor_mul(t2[0:32, :], xo, sinl_sb[32:64, :])
                            nc.vector.tensor_sub(
                                anrm[0:32, MT - 1, :], t1[0:32, :], t2[0:32, :]
                            )
                            nc.vector.tensor_mul(t1[32:64, :], xo, cosl_sb[32:64, :])
                            nc.vector.tensor_mul(t2[32:64, :], xe, sinl_sb[0:32, :])
                            nc.vector.tensor_add(
                                anrm[32:64, MT - 1, :], t1[32:64, :], t2[32:64, :]
                            )
                            nc.vector.memset(anrm[64:128, MT - 1, :], 0.0)
                            nc.sync.dma_start(
                                locb[QLC:MT].rearrange("m p t -> p m t"),
                                anrm[:, QLC:MT, :],
                            )
                            if not single:
                                nc.gpsimd.collective_compute(
                                    "AllGather",
                                    ALU.bypass,
                                    replica_groups=[list(range(NCORE))],
                                    ins=[locb[QLC:MT].opt()],
                                    outs=[gathkv.opt()],
                                )
                            # kv gather-in + whole kv b-projection — overlaps
                            # the q-group a-proj matmuls still streaming on PE
                            nc.sync.dma_start(wkvb_sb[:], wkvb)
                            for mm in range(KLC):
                                nc.sync.dma_start(
                                    kag[:, mm],
                                    gathkv[:, mm].rearrange("c p t -> p c t"),
                                )
                            for half in range(2):
                                nc.sync.dma_start(
                                    kpe[half * ROPE : (half + 1) * ROPE],
                                    gathkv[:, NKV - 1, 0:ROPE, :].rearrange(
                                        "c p t -> p c t"
                                    ),
                                )
                            # phase-2 weights load here: PE is busy with the
                            # q-group a-proj, DMA queues are otherwise idle
                            nc.sync.dma_start(wqb_sb[:], wqb)
                            nc.sync.dma_start(wo_sb[:], wo)
                            # k_nope^T per head: [128, T]
                            for hh in range(HPC):
                                for s in range(4):
                                    psb = ps1.tile([P, 512], F32, tag="bp")
                                    for k in range(KLC):
                                        nc.tensor.matmul(
                                            psb[:],
                                            wkvb_sb[:, k, hh * 128 : (hh + 1) * 128],
                                            kag_f[:, k, s * 512 : (s + 1) * 512],
                                            start=(k == 0),
                                            stop=(k == KLC - 1),
                                        )
                                    nc.scalar.copy(
                                        kTn[:, hh, s * 512 : (s + 1) * 512], psb[:]
                                    )
                            # v token-major: [t, 2*VD] per token tile
                            for tt in range(NT):
                                psb = ps1.tile([P, 512], F32, tag="bp")
                                for k in range(KLC):
                                    nc.tensor.matmul(
                                        psb[:, 0 : HPC * VD],
                                        kag[
                                            :, k, tt // 2,
                                            (tt % 2) * 128 : (tt % 2) * 128 + 128,
                                        ],
                                        wkvb_sb[:, k, 256:512],
                                        start=(k == 0),
                                        stop=(k == KLC - 1),
                                    )
                                nc.vector.tensor_copy(
                                    vtok[:, tt, :], psb[:, 0 : HPC * VD]
                                )

                    # q group: normalize + ship
                    rsq_q = p1.tile([P, TLOC], F32, tag="rsq_q")
                    tmpf = p1.tile([P, TLOC], F32, tag="tmpf")
                    nc.scalar.activation(
                        tmpf[:], ssq[:], AF.Sqrt, bias=eps_sb[:], scale=1.0 / QL
                    )
                    nc.vector.reciprocal(rsq_q[:], tmpf[:])
                    for m in range(QLC):
                        nc.vector.tensor_mul(anrm[:, m, :], araw[:, m, :], rsq_q[:])
                    nc.sync.dma_start(
                        locb[0:QLC].rearrange("m p t -> p m t"), anrm[:, 0:QLC, :]
                    )
                    if not single:
                        nc.gpsimd.collective_compute(
                            "AllGather",
                            ALU.bypass,
                            replica_groups=[list(range(NCORE))],
                            ins=[locb[0:QLC].opt()],
                            outs=[gathq.opt()],
                        )

                # ---------------- phase 2: q b-proj + attention + w_o ----------
                with (
                    tc.tile_pool(name="p2", bufs=1) as p2,
                    tc.tile_pool(name="ptp", bufs=2) as ptp,
                    tc.tile_pool(name="rcp", bufs=2) as rcp,
                    tc.tile_pool(name="ps2", bufs=2, space="PSUM") as ps2,
                    tc.tile_pool(name="psA", bufs=2, space="PSUM") as psA,
                ):
                    qag = p2.tile([P, QLC, NCORE, TLOC], BF16, tag="qag")
                    for m in range(QLC):
                        nc.sync.dma_start(
                            qag[:, m], gathq[:, m].rearrange("c p t -> p c t")
                        )
                    qag_f = qag.rearrange("p m c t -> p m (c t)")

                    # q^T: nope [128, T] per head; both heads' pe packed M=128
                    # (wqb cols: [h0 nope | h1 nope | h0 pe | h1 pe])
                    for hh in range(HPC):
                        for s in range(4):
                            ps = ps2.tile([P, 512], F32, tag="bp")
                            for k in range(QLC):
                                nc.tensor.matmul(
                                    ps[:],
                                    wqb_sb[:, k, hh * NOPE : (hh + 1) * NOPE],
                                    qag_f[:, k, s * 512 : (s + 1) * 512],
                                    start=(k == 0),
                                    stop=(k == QLC - 1),
                                )
                            nc.scalar.copy(qTn[:, hh, s * 512 : (s + 1) * 512], ps[:])
                    for s in range(4):
                        ps = ps2.tile([P, 512], F32, tag="bp")
                        for k in range(QLC):
                            nc.tensor.matmul(
                                ps[:],
                                wqb_sb[:, k, HPC * NOPE : HPC * QK],
                                qag_f[:, k, s * 512 : (s + 1) * 512],
                                start=(k == 0),
                                stop=(k == QLC - 1),
                            )
                        # rope both heads' pe straight out of PSUM
                        # (PSUM x SBUF ops are exempt from the equal-base rule)
                        sl = slice(s * 512, (s + 1) * 512)
                        cs, sn = cosf_sb[:, sl], sinf_sb[:, sl]
                        rt = p2.tile([P, 512], BF16, tag="rt")
                        for hh in range(HPC):
                            b = hh * ROPE
                            xe, xo = ps[b : b + 32, :], ps[b + 32 : b + 64, :]
                            nc.vector.tensor_mul(qTp[b : b + 32, sl], xe, cs)
                            nc.vector.tensor_mul(rt[b : b + 32, :], xo, sn)
                            nc.vector.tensor_sub(
                                qTp[b : b + 32, sl],
                                qTp[b : b + 32, sl],
                                rt[b : b + 32, :],
                            )
                            nc.vector.tensor_mul(qTp[b + 32 : b + 64, sl], xo, cs)
                            nc.vector.tensor_mul(rt[b + 32 : b + 64, :], xe, sn)
                            nc.vector.tensor_add(
                                qTp[b + 32 : b + 64, sl],
                                qTp[b + 32 : b + 64, sl],
                                rt[b + 32 : b + 64, :],
                            )

                    # attention (S^T layout, no max-subtraction) + w_o per q-slice
                    for qs in range(NQS):
                        for hh in range(HPC):
                            nk = 4 * qs + 4
                            PT = ptp.tile([P, NT, 512], BF16, tag="PT")
                            for kt in range(nk):
                                # columns 0..r*128 are fully causal-masked:
                                # skip them in every matmul of this k-tile
                                r = kt - 4 * qs
                                c0 = max(r, 0) * 128
                                ps_s = psA.tile([P, 512], F32, tag="ps_s")
                                nc.tensor.matmul(
                                    ps_s[:, c0:512],
                                    kTn[:, hh, kt * 128 : (kt + 1) * 128],
                                    qTn[:, hh, qs * 512 + c0 : (qs + 1) * 512],
                                    start=True,
                                    stop=False,
                                )
                                hb = hh * ROPE
                                nc.tensor.matmul(
                                    ps_s[:, c0:512],
                                    kpe[
                                        hb : hb + ROPE, kt // 2,
                                        (kt % 2) * 128 : (kt % 2) * 128 + 128,
                                    ],
                                    qTp[hb : hb + ROPE, qs * 512 + c0 : (qs + 1) * 512],
                                    start=False,
                                    stop=True,
                                )
                                if c0 > 0:
                                    nc.vector.memset(PT[:, kt, 0:c0], 0.0)
                                nc.scalar.activation(
                                    PT[:, kt, c0:512], ps_s[:, c0:512], AF.Exp
                                )
                                if 0 <= r <= 3:
                                    nc.vector.tensor_mul(
                                        PT[:, kt, r * 128 : (r + 1) * 128],
                                        PT[:, kt, r * 128 : (r + 1) * 128],
                                        tri_sb[:],
                                    )
                            ps_ow = ps2.tile([P, 1024], F32, tag="bpw")
                            ps_o = ps_ow[:, 0:512]
                            for kt in range(nk):
                                c0 = max(kt - 4 * qs, 0) * 128
                                nc.tensor.matmul(
                                    ps_o[:, c0:512],
                                    vtok[:, kt, hh * VD : (hh + 1) * VD],
                                    PT[:, kt, c0:512],
                                    start=(kt == 0),
                                    stop=(kt == nk - 1),
                                )
                            ps_d = ps2.tile([P, 512], F32, tag="bp")
                            for kt in range(nk):
                                c0 = max(kt - 4 * qs, 0) * 128
                                nc.tensor.matmul(
                                    ps_d[:, c0:512],
                                    ones_sb[:],
                                    PT[:, kt, c0:512],
                                    start=(kt == 0),
                                    stop=(kt == nk - 1),
                                )
                            rec = rcp.tile([P, 512], F32, tag="rec")
                            nc.vector.reciprocal(rec[:], ps_d[:])
                            nc.vector.tensor_mul(
                                OnT[:, hh, qs * 512 : (qs + 1) * 512], ps_o[:], rec[:]
                            )
                        for tt in range(4 * qs, 4 * qs + 4):
                            for hp in range(NHS // 2):  # paired 1024-col slices
                                ps_f = ps2.tile([P, 1024], F32, tag="bpw")
                                for half in range(2):
                                    hs = hp * 2 + half
                                    c = slice(half * 512, half * 512 + 512)
                                    nc.tensor.matmul(
                                        ps_f[:, c],
                                        OnT[:, 0, tt * 128 : (tt + 1) * 128],
                                        wo_sb[:, 0, hs * 512 : (hs + 1) * 512],
                                        start=True,
                                        stop=False,
                                    )
                                    nc.tensor.matmul(
                                        ps_f[:, c],
                                        OnT[:, 1, tt * 128 : (tt + 1) * 128],
                                        wo_sb[:, 1, hs * 512 : (hs + 1) * 512],
                                        start=False,
                                        stop=True,
                                    )
                                oc = ocp.tile([P, 1024], BF16, tag="oc")
                                # alternate copy engine so copies keep pace
                                # with the four matmuls per pair
                                if hp % 2 == 0:
                                    nc.scalar.copy(oc[:], ps_f[:])
                                else:
                                    nc.vector.tensor_copy(oc[:], ps_f[:])
                                nc.sync.dma_start(
                                    out[
                                        tt * 128 : (tt + 1) * 128,
                                        hp * 1024 : (hp + 1) * 1024,
                                    ],
                                    oc[:],
                                )

    nc.compile()
    return nc


_NC_CACHE = None


def _get_nc():
    global _NC_CACHE
    if _NC_CACHE is None:
        _NC_CACHE = _build_nc()
    return _NC_CACHE


def _bf(x):
    return np.ascontiguousarray(x.astype(BF))


def _prep_in_maps(inputs):
    hidden = np.asarray(inputs["hidden_states"], dtype=np.float32)
    w_q_a = np.asarray(inputs["w_q_a"], dtype=np.float32)
    q_a_norm_w = np.asarray(inputs["q_a_norm_w"], dtype=np.float32)
    w_q_b = np.asarray(inputs["w_q_b"], dtype=np.float32)
    w_kv_a = np.asarray(inputs["w_kv_a"], dtype=np.float32)
    kv_a_norm_w = np.asarray(inputs["kv_a_norm_w"], dtype=np.float32)
    w_kv_b = np.asarray(inputs["w_kv_b"], dtype=np.float32)
    w_o = np.asarray(inputs["w_o"], dtype=np.float32)
    pos = np.asarray(inputs["positions"]).astype(np.float32)

    # rope tables, feature-major, evens/odds share the same row index
    inv_freq = _yarn_inv_freq()
    freqs = pos[:, None] * inv_freq[None, :]          # [T, 32]
    cosf = np.cos(freqs).T * COS_SIN_MSCALE           # [32, T]
    sinf = np.sin(freqs).T * COS_SIN_MSCALE
    cosf_b, sinf_b = _bf(cosf), _bf(sinf)
    cosl2 = np.concatenate([cosf_b, cosf_b], 0)       # duplicated halves
    sinl2 = np.concatenate([sinf_b, sinf_b], 0)

    # a-proj weights: [17 mtiles, 128p, 40k, 128c], pe cols de-interleaved
    wkva_pe = w_kv_a[:, KL:][:, PE_PERM]
    wa_full = np.concatenate(
        [w_q_a, w_kv_a[:, :KL], wkva_pe, np.zeros((HID, 64), np.float32)], axis=1
    )  # [5120, 2176]
    wa_l = _bf(wa_full.reshape(HCH, P, MT, P).transpose(2, 1, 0, 3))

    # fold RMSNorm gains + attention scale into b-proj weights
    wqb_s = w_q_b * q_a_norm_w[:, None] * ATTN_SCALE
    wkvb_s = w_kv_b * kv_a_norm_w[:, None]

    ones_b = _bf(np.ones((P, P), np.float32))
    tri_b = _bf(np.triu(np.ones((P, P), np.float32)))

    shared = {
        "wa": wa_l,
        "cosf": cosf_b,
        "sinf": sinf_b,
        "ones": ones_b,
        "tri": tri_b,
    }

    in_maps = []
    for c in range(NCORE):
        h0 = HPC * c
        # hidden slice, feature-major [128, 40, 256]
        hs = hidden[c * TLOC : (c + 1) * TLOC, :]
        hT_l = _bf(hs.T.reshape(HCH, P, TLOC).transpose(1, 0, 2))
        # w_q_b cols for this core's heads: [h0 nope | h1 nope | h0 pe | h1 pe]
        nope_cols, pe_cols = [], []
        for h in range(h0, h0 + HPC):
            blk = wqb_s[:, h * QK : (h + 1) * QK]
            nope_cols.append(blk[:, :NOPE])
            pe_cols.append(blk[:, NOPE:][:, PE_PERM])
        wqb_core = np.concatenate(nope_cols + pe_cols, axis=1)  # [1536, 384]
        wqb_l = _bf(wqb_core.reshape(QLC, P, HPC * QK).transpose(1, 0, 2))
        # w_kv_b cols: [h0 nope, h1 nope, h0 v, h1 v]
        nopes = [
            wkvb_s[:, h * (NOPE + VD) : h * (NOPE + VD) + NOPE]
            for h in range(h0, h0 + HPC)
        ]
        vs = [
            wkvb_s[:, h * (NOPE + VD) + NOPE : (h + 1) * (NOPE + VD)]
            for h in range(h0, h0 + HPC)
        ]
        wkvb_core = np.concatenate(nopes + vs, axis=1)  # [512, 512]
        wkvb_l = _bf(wkvb_core.reshape(KLC, P, 512).transpose(1, 0, 2))
        # w_o rows for this core's heads: [128, 2, 5120]
        wo_core = w_o[h0 * VD : (h0 + HPC) * VD, :]
        wo_l = _bf(wo_core.reshape(HPC, P, HID).transpose(1, 0, 2))

        m = dict(shared)
        m.update(
            {
                "hT": hT_l,
                "wqb": wqb_l,
                "wkvb": wkvb_l,
                "wo": wo_l,
                "cosl": np.ascontiguousarray(cosl2[:, c * TLOC : (c + 1) * TLOC]),
                "sinl": np.ascontiguousarray(sinl2[:, c * TLOC : (c + 1) * TLOC]),
            }
        )
        in_maps.append(m)
    return in_maps


def kernel(**inputs):
    global LAST_EXEC_NS
    nc = _get_nc()
    in_maps = _prep_in_maps(inputs)
    trace = os.environ.get("KERNEL_TRACE", "0") == "1"
    res = run_bass_kernel_spmd(
        nc, in_maps, core_ids=list(range(NCORE)), trace=trace
    )
    LAST_EXEC_NS = res.exec_time_ns
    out = res.results[0]["out"].astype(np.float32)
    for i in range(1, NCORE):
        out += res.results[i]["out"].astype(np.float32)
    return out



# revision 8
# speedup vs baseline: 1.0334x; 1.0334x over previous
"""DeepSeek MLA prefill on 8 TRN2 NeuronCores.

Sharding: tensor-parallel over heads (2 heads/core) for the b-projections,
attention and w_o (row-parallel -> host sums partials); sequence-parallel
a-projections (each core computes 256 tokens of q_a/kv_a/k_pe, normalizes,
ropes k_pe, then on-device AllGathers replicate the 2112x256 activations).
The kv-group a-proj runs first so its (small) gather and the whole kv
b-projection overlap the q-group a-proj and gather.

All activations that feed matmuls are kept feature-major ([d, T]) so no
on-device transposes are needed; v is produced token-major directly.
Matmuls run in bf16 with f32 PSUM accumulation (rel-err gate is ~2e-2).
"""

import math
import os

import ml_dtypes
import numpy as np

import concourse.bacc as bacc
import concourse.bass_isa as bass_isa
import concourse.mybir as mybir
import concourse.tile as tile
from concourse.bass_utils import run_bass_kernel_spmd

F32 = mybir.dt.float32
BF16 = mybir.dt.bfloat16
AF = mybir.ActivationFunctionType
ALU = mybir.AluOpType

# problem dims (hardcoded per contract)
T, HID, H = 2048, 5120, 16
QL, KL = 1536, 512
NOPE, ROPE, VD = 128, 64, 128
QK = NOPE + ROPE
EPS = 1e-6
NCORE = 8
HPC = H // NCORE          # heads per core = 2
TLOC = T // NCORE         # tokens per core = 256
P = 128
HCH = HID // P            # 40 hidden chunks
QLC = QL // P             # 12
KLC = KL // P             # 4
MT = QLC + KLC + 1        # 17 a-proj output tiles (12 q + 4 kv + 1 pe[64])
NKV = MT - QLC            # 5 kv-group tiles
NT = T // P               # 16 token tiles
NQS = 4                   # 512-wide q slices per head
NHS = HID // 512          # 10 output column slices

# yarn rope params
BASE, FACTOR = 10000.0, 40.0
BETA_FAST, BETA_SLOW, ORIG_MAX = 32.0, 1.0, 4096
MSCALE = 1.0
MSCALE_ALL_DIM = 1.0


def _yarn_get_mscale(scale, m):
    if scale <= 1.0:
        return 1.0
    return 0.1 * m * math.log(scale) + 1.0


def _yarn_inv_freq():
    pos_freqs = BASE ** (np.arange(0, ROPE, 2, dtype=np.float64) / ROPE)
    extra = 1.0 / pos_freqs
    inter = 1.0 / (FACTOR * pos_freqs)

    def corr_dim(n):
        return ROPE * math.log(ORIG_MAX / (n * 2 * math.pi)) / (2 * math.log(BASE))

    low = max(math.floor(corr_dim(BETA_FAST)), 0)
    high = min(math.ceil(corr_dim(BETA_SLOW)), ROPE - 1)
    ramp = np.clip(
        (np.arange(ROPE // 2, dtype=np.float64) - low) / max(high - low, 0.001),
        0.0,
        1.0,
    )
    mask = 1.0 - ramp
    return (inter * (1.0 - mask) + extra * mask).astype(np.float32)


COS_SIN_MSCALE = _yarn_get_mscale(FACTOR, MSCALE) / _yarn_get_mscale(
    FACTOR, MSCALE_ALL_DIM
)
_M = _yarn_get_mscale(FACTOR, MSCALE_ALL_DIM)
ATTN_SCALE = (QK ** -0.5) * _M * _M

BF = ml_dtypes.bfloat16
# de-interleave perm: even rope dims then odd rope dims
PE_PERM = np.concatenate([np.arange(0, ROPE, 2), np.arange(1, ROPE, 2)])

LAST_EXEC_NS = None


def _build_nc(single=False, reps=1):
    # single=True: no collective, 1 core — for cost-model timeline sims only
    nc = bacc.Bacc(
        "TRN2",
        target_bir_lowering=False,
        debug=False,
        num_devices=1 if single else NCORE,
    )

    hT = nc.dram_tensor("hT", [P, HCH, TLOC], BF16, kind="ExternalInput").ap()
    wa = nc.dram_tensor("wa", [MT, P, HCH, P], BF16, kind="ExternalInput").ap()
    wqb = nc.dram_tensor("wqb", [P, QLC, HPC * QK], BF16, kind="ExternalInput").ap()
    wkvb = nc.dram_tensor("wkvb", [P, KLC, 512], BF16, kind="ExternalInput").ap()
    wo = nc.dram_tensor("wo", [P, HPC, HID], BF16, kind="ExternalInput").ap()
    cosf = nc.dram_tensor("cosf", [ROPE // 2, T], BF16, kind="ExternalInput").ap()
    sinf = nc.dram_tensor("sinf", [ROPE // 2, T], BF16, kind="ExternalInput").ap()
    cosl = nc.dram_tensor("cosl", [ROPE, TLOC], BF16, kind="ExternalInput").ap()
    sinl = nc.dram_tensor("sinl", [ROPE, TLOC], BF16, kind="ExternalInput").ap()
    onesd = nc.dram_tensor("ones", [P, P], BF16, kind="ExternalInput").ap()
    trid = nc.dram_tensor("tri", [P, P], BF16, kind="ExternalInput").ap()
    out = nc.dram_tensor("out", [T, HID], BF16, kind="ExternalOutput").ap()

    locb = nc.dram_tensor("locb", [MT, P, TLOC], BF16).ap()
    gathkv = nc.dram_tensor(
        "gathkv", [NCORE, NKV, P, TLOC], BF16, addr_space="Shared"
    ).ap()
    gathq = nc.dram_tensor(
        "gathq", [NCORE, QLC, P, TLOC], BF16, addr_space="Shared"
    ).ap()

    with tile.TileContext(nc) as tc:
        with (
            tc.tile_pool(name="const", bufs=1) as cp,
            tc.tile_pool(name="persist", bufs=1) as pp,
            tc.tile_pool(name="ocp", bufs=2) as ocp,
        ):
            ones_sb = cp.tile([P, P], BF16, tag="ones")
            tri_sb = cp.tile([P, P], BF16, tag="tri")
            cosf_sb = cp.tile([ROPE // 2, T], BF16, tag="cosf")
            sinf_sb = cp.tile([ROPE // 2, T], BF16, tag="sinf")
            cosl_sb = cp.tile([ROPE, TLOC], BF16, tag="cosl")
            sinl_sb = cp.tile([ROPE, TLOC], BF16, tag="sinl")
            eps_sb = cp.tile([P, 1], F32, tag="eps")
            nc.vector.memset(eps_sb[:], EPS)

            # persistent attention operands (live across the phase transition)
            qTn = pp.tile([P, HPC, T], BF16, tag="qTn")
            # both heads' roped q_pe packed: rows [h0e h0o h1e h1o] x 32
            qTp = pp.tile([P, T], BF16, tag="qTp")
            kTn = pp.tile([P, HPC, T], BF16, tag="kTn")
            vtok = pp.tile([P, NT, HPC * VD], BF16, tag="vtok")
            OnT = pp.tile([P, HPC, T], BF16, tag="OnT")
            kag = pp.tile([P, KLC, NCORE, TLOC], BF16, tag="kag")
            # k_pe duplicated into both 64-row halves so each head's score
            # matmul has lhsT/rhs at the same base partition (0 or 64)
            kpe = pp.tile([P, NCORE, TLOC], BF16, tag="kpe")
            wkvb_sb = pp.tile([P, KLC, 512], BF16, tag="wkvb")
            wqb_sb = pp.tile([P, QLC, HPC * QK], BF16, tag="wqb")
            wo_sb = pp.tile([P, HPC, HID], BF16, tag="wo")
            kag_f = kag.rearrange("p m c t -> p m (c t)")

            for _rep in range(reps):
                # ---------------- phase 1: local a-projections ----------------
                with (
                    tc.tile_pool(name="p1", bufs=1) as p1,
                    tc.tile_pool(name="wap", bufs=3) as wap,
                    tc.tile_pool(name="sqp", bufs=3) as sqp,
                    tc.tile_pool(name="ps1", bufs=3, space="PSUM") as ps1,
                    tc.tile_pool(name="pss", bufs=1, space="PSUM") as pss,
                ):
                    hT_sb = p1.tile([P, HCH, TLOC], BF16, tag="hT")
                    # only the first tiny chunk up front; the rest is
                    # interleaved with the first weight tile's chunks below
                    nc.sync.dma_start(hT_sb[:, 0:2, :], hT[:, 0:2, :])
                    araw = p1.tile([P, MT, TLOC], BF16, tag="araw")
                    anrm = p1.tile([P, MT, TLOC], BF16, tag="anrm")
                    ssq = pss.tile([P, TLOC], F32, tag="ssq")
                    sskv = pss.tile([P, TLOC], F32, tag="sskv")

                    # kv-group mtiles first so their collective + the whole kv
                    # b-projection overlap the (3x bigger) q-group a-proj
                    for m in list(range(QLC, MT)) + list(range(QLC)):
                        wt = wap.tile([P, HCH, P], BF16, tag="wt")
                        if m == QLC:  # first mtile: pace-matched interleave of
                            # weight chunks and the rest of hT so the k-loop
                            # never waits long on either stream
                            nc.sync.dma_start(wt[:, 0:5, :], wa[m, :, 0:5, :])
                            nc.sync.dma_start(ones_sb[:], onesd)
                            nc.sync.dma_start(cosl_sb[:], cosl)
                            nc.sync.dma_start(sinl_sb[:], sinl)
                            for (w0, w1), (h0, h1) in [
                                ((5, 14), (2, 8)),
                                ((14, 24), (8, 16)),
                                ((24, 40), (16, 28)),
                                ((40, 40), (28, 40)),
                            ]:
                                if w1 > w0:
                                    nc.sync.dma_start(
                                        wt[:, w0:w1, :], wa[m, :, w0:w1, :]
                                    )
                                nc.sync.dma_start(hT_sb[:, h0:h1, :], hT[:, h0:h1, :])
                        else:
                            nc.sync.dma_start(wt[:], wa[m])
                            if m == QLC - 2:
                                # phase-2 q weights: late enough to not crowd
                                # the wa stream, early enough for phase 2
                                nc.sync.dma_start(wqb_sb[:], wqb)
                        ps = ps1.tile([P, TLOC], F32, tag="aps")
                        for k in range(HCH):
                            nc.tensor.matmul(
                                ps[:],
                                wt[:, k, :],
                                hT_sb[:, k, :],
                                start=(k == 0),
                                stop=(k == HCH - 1),
                            )
                        nc.scalar.copy(araw[:, m, :], ps[:])
                        if m < QLC + KLC:
                            sq = sqp.tile([P, TLOC], BF16, tag="sq")
                            nc.scalar.activation(sq[:], ps[:], AF.Square)
                            if m < QLC:
                                nc.tensor.matmul(
                                    ssq[:],
                                    ones_sb[:],
                                    sq[:],
                                    start=(m == 0),
                                    stop=(m == QLC - 1),
                                    skip_group_check=True,
                                )
                            else:
                                nc.tensor.matmul(
                                    sskv[:],
                                    ones_sb[:],
                                    sq[:],
                                    start=(m == QLC),
                                    stop=(m == QLC + KLC - 1),
                                    skip_group_check=True,
                                )

                        if m == MT - 1:
                            # kv group locally complete: normalize, rope, ship
                            rsq_k = p1.tile([P, TLOC], F32, tag="rsq_k")
                            tmpf2 = p1.tile([P, TLOC], F32, tag="tmpf2")
                            nc.scalar.activation(
                                tmpf2[:], sskv[:], AF.Sqrt,
                                bias=eps_sb[:], scale=1.0 / KL,
                            )
                            nc.vector.reciprocal(rsq_k[:], tmpf2[:])
                            for mm in range(QLC, QLC + KLC):
                                nc.vector.tensor_mul(
                                    anrm[:, mm, :], araw[:, mm, :], rsq_k[:]
                                )
                            # rope k_pe (rows 0:32 even, 32:64 odd of tile MT-1).
                            # Two-SBUF-input ops must share base partition, so
                            # cos/sin tables are duplicated across both halves.
                            t1 = p1.tile([ROPE, TLOC], BF16, tag="t1")
                            t2 = p1.tile([ROPE, TLOC], BF16, tag="t2")
                            xe = araw[0:32, MT - 1, :]
                            xo = araw[32:64, MT - 1, :]
                            nc.vector.tensor_mul(t1[0:32, :], xe, cosl_sb[0:32, :])
                            nc.vector.tensor_mul(t2[0:32, :], xo, sinl_sb[32:64, :])
                            nc.vector.tensor_sub(
                                anrm[0:32, MT - 1, :], t1[0:32, :], t2[0:32, :]
                            )
                            nc.vector.tensor_mul(t1[32:64, :], xo, cosl_sb[32:64, :])
                            nc.vector.tensor_mul(t2[32:64, :], xe, sinl_sb[0:32, :])
                            nc.vector.tensor_add(
                                anrm[32:64, MT - 1, :], t1[32:64, :], t2[32:64, :]
                            )
                            nc.vector.memset(anrm[64:128, MT - 1, :], 0.0)
                            nc.sync.dma_start(
                                locb[QLC:MT].rearrange("m p t -> p m t"),
                                anrm[:, QLC:MT, :],
                            )
                            if not single:
                                nc.gpsimd.collective_compute(
                                    "AllGather",
                                    ALU.bypass,
                                    replica_groups=[list(range(NCORE))],
                                    ins=[locb[QLC:MT].opt()],
                                    outs=[gathkv.opt()],
                                )
                            # kv gather-in + whole kv b-projection — overlaps
                            # the q-group a-proj matmuls still streaming on PE
                            nc.sync.dma_start(wkvb_sb[:], wkvb)
                            for mm in range(KLC):
                                nc.sync.dma_start(
                                    kag[:, mm],
                                    gathkv[:, mm].rearrange("c p t -> p c t"),
                                )
                            for half in range(2):
                                nc.sync.dma_start(
                                    kpe[half * ROPE : (half + 1) * ROPE],
                                    gathkv[:, NKV - 1, 0:ROPE, :].rearrange(
                                        "c p t -> p c t"
                                    ),
                                )
                            # k_nope^T per head: [128, T]
                            for hh in range(HPC):
                                for s in range(4):
                                    psb = ps1.tile([P, 512], F32, tag="bp")
                                    for k in range(KLC):
                                        nc.tensor.matmul(
                                            psb[:],
                                            wkvb_sb[:, k, hh * 128 : (hh + 1) * 128],
                                            kag_f[:, k, s * 512 : (s + 1) * 512],
                                            start=(k == 0),
                                            stop=(k == KLC - 1),
                                        )
                                    nc.scalar.copy(
                                        kTn[:, hh, s * 512 : (s + 1) * 512], psb[:]
                                    )
                            # v token-major: [t, 2*VD] per token tile
                            for tt in range(NT):
                                psb = ps1.tile([P, 512], F32, tag="bp")
                                for k in range(KLC):
                                    nc.tensor.matmul(
                                        psb[:, 0 : HPC * VD],
                                        kag[
                                            :, k, tt // 2,
                                            (tt % 2) * 128 : (tt % 2) * 128 + 128,
                                        ],
                                        wkvb_sb[:, k, 256:512],
                                        start=(k == 0),
                                        stop=(k == KLC - 1),
                                    )
                                nc.vector.tensor_copy(
                                    vtok[:, tt, :], psb[:, 0 : HPC * VD]
                                )

                    # q group: normalize + ship
                    rsq_q = p1.tile([P, TLOC], F32, tag="rsq_q")
                    tmpf = p1.tile([P, TLOC], F32, tag="tmpf")
                    nc.scalar.activation(
                        tmpf[:], ssq[:], AF.Sqrt, bias=eps_sb[:], scale=1.0 / QL
                    )
                    nc.vector.reciprocal(rsq_q[:], tmpf[:])
                    for m in range(QLC):
                        nc.vector.tensor_mul(anrm[:, m, :], araw[:, m, :], rsq_q[:])
                    nc.sync.dma_start(
                        locb[0:QLC].rearrange("m p t -> p m t"), anrm[:, 0:QLC, :]
                    )
                    if not single:
                        nc.gpsimd.collective_compute(
                            "AllGather",
                            ALU.bypass,
                            replica_groups=[list(range(NCORE))],
                            ins=[locb[0:QLC].opt()],
                            outs=[gathq.opt()],
                        )

                # ---------------- phase 2: q b-proj + attention + w_o ----------
                with (
                    tc.tile_pool(name="p2", bufs=1) as p2,
                    tc.tile_pool(name="ptp", bufs=2) as ptp,
                    tc.tile_pool(name="rcp", bufs=2) as rcp,
                    tc.tile_pool(name="dsp", bufs=2) as dsp,
                    tc.tile_pool(name="psB", bufs=3, space="PSUM") as psB,
                    tc.tile_pool(name="psA", bufs=3, space="PSUM") as psA,
                ):
                    qag = p2.tile([P, QLC, NCORE, TLOC], BF16, tag="qag")
                    for m in range(QLC):
                        nc.sync.dma_start(
                            qag[:, m], gathq[:, m].rearrange("c p t -> p c t")
                        )
                        if m == 5:
                            # rope tables needed once the pe slices start
                            nc.sync.dma_start(cosf_sb[:], cosf)
                            nc.sync.dma_start(sinf_sb[:], sinf)
                    nc.sync.dma_start(tri_sb[:], trid)
                    nc.sync.dma_start(wo_sb[:], wo)
                    qag_f = qag.rearrange("p m c t -> p m (c t)")

                    # q^T: nope [128, T] per head; both heads' pe packed M=128
                    # (wqb cols: [h0 nope | h1 nope | h0 pe | h1 pe])
                    for hh in range(HPC):
                        for s in range(4):
                            ps = psB.tile([P, 512], F32, tag="bp")
                            for k in range(QLC):
                                nc.tensor.matmul(
                                    ps[:],
                                    wqb_sb[:, k, hh * NOPE : (hh + 1) * NOPE],
                                    qag_f[:, k, s * 512 : (s + 1) * 512],
                                    start=(k == 0),
                                    stop=(k == QLC - 1),
                                )
                            nc.scalar.copy(qTn[:, hh, s * 512 : (s + 1) * 512], ps[:])

                    def q_pe_slice(s):
                        ps = psB.tile([P, 512], F32, tag="bp")
                        for k in range(QLC):
                            nc.tensor.matmul(
                                ps[:],
                                wqb_sb[:, k, HPC * NOPE : HPC * QK],
                                qag_f[:, k, s * 512 : (s + 1) * 512],
                                start=(k == 0),
                                stop=(k == QLC - 1),
                            )
                        # rope both heads' pe straight out of PSUM
                        # (PSUM x SBUF ops are exempt from the equal-base rule)
                        sl = slice(s * 512, (s + 1) * 512)
                        cs, sn = cosf_sb[:, sl], sinf_sb[:, sl]
                        rt = p2.tile([P, 512], BF16, tag="rt")
                        for hh in range(HPC):
                            b = hh * ROPE
                            xe, xo = ps[b : b + 32, :], ps[b + 32 : b + 64, :]
                            nc.vector.tensor_mul(qTp[b : b + 32, sl], xe, cs)
                            nc.vector.tensor_mul(rt[b : b + 32, :], xo, sn)
                            nc.vector.tensor_sub(
                                qTp[b : b + 32, sl],
                                qTp[b : b + 32, sl],
                                rt[b : b + 32, :],
                            )
                            nc.vector.tensor_mul(qTp[b + 32 : b + 64, sl], xo, cs)
                            nc.vector.tensor_mul(rt[b + 32 : b + 64, :], xe, sn)
                            nc.vector.tensor_add(
                                qTp[b + 32 : b + 64, sl],
                                qTp[b + 32 : b + 64, sl],
                                rt[b + 32 : b + 64, :],
                            )

                    # attention (S^T layout, no max-subtraction); softmax
                    # denominator accumulated on DVE + Pool partition-reduce
                    # instead of PE ones-matmuls
                    def attn(qs):
                        for hh in range(HPC):
                            nk = 4 * qs + 4
                            PT = ptp.tile([P, NT, 512], BF16, tag="PT")
                            dsum = dsp.tile([P, 512], F32, tag="dsum")
                            for kt in range(nk):
                                # columns 0..r*128 are fully causal-masked:
                                # skip them in every matmul of this k-tile
                                r = kt - 4 * qs
                                c0 = max(r, 0) * 128
                                ps_s = psA.tile([P, 512], F32, tag="ps_s")
                                nc.tensor.matmul(
                                    ps_s[:, c0:512],
                                    kTn[:, hh, kt * 128 : (kt + 1) * 128],
                                    qTn[:, hh, qs * 512 + c0 : (qs + 1) * 512],
                                    start=True,
                                    stop=False,
                                )
                                hb = hh * ROPE
                                nc.tensor.matmul(
                                    ps_s[:, c0:512],
                                    kpe[
                                        hb : hb + ROPE, kt // 2,
                                        (kt % 2) * 128 : (kt % 2) * 128 + 128,
                                    ],
                                    qTp[hb : hb + ROPE, qs * 512 + c0 : (qs + 1) * 512],
                                    start=False,
                                    stop=True,
                                )
                                if c0 > 0:
                                    nc.vector.memset(PT[:, kt, 0:c0], 0.0)
                                nc.scalar.activation(
                                    PT[:, kt, c0:512], ps_s[:, c0:512], AF.Exp
                                )
                                if 0 <= r <= 3:
                                    nc.vector.tensor_mul(
                                        PT[:, kt, r * 128 : (r + 1) * 128],
                                        PT[:, kt, r * 128 : (r + 1) * 128],
                                        tri_sb[:],
                                    )
                                if kt == 0:
                                    nc.gpsimd.tensor_copy(dsum[:], PT[:, 0, :])
                                else:
                                    nc.gpsimd.tensor_add(
                                        dsum[:], dsum[:], PT[:, kt, :]
                                    )
                            ps_o = psB.tile([P, 512], F32, tag="bp")
                            for kt in range(nk):
                                c0 = max(kt - 4 * qs, 0) * 128
                                nc.tensor.matmul(
                                    ps_o[:, c0:512],
                                    vtok[:, kt, hh * VD : (hh + 1) * VD],
                                    PT[:, kt, c0:512],
                                    start=(kt == 0),
                                    stop=(kt == nk - 1),
                                )
                            dred = dsp.tile([P, 512], F32, tag="dred")
                            nc.gpsimd.partition_all_reduce(
                                dred[:], dsum[:], channels=128,
                                reduce_op=bass_isa.ReduceOp.add,
                            )
                            rec = rcp.tile([P, 512], F32, tag="rec")
                            nc.vector.reciprocal(rec[:], dred[:])
                            nc.vector.tensor_mul(
                                OnT[:, hh, qs * 512 : (qs + 1) * 512], ps_o[:], rec[:]
                            )

                    def wo_block(qs):
                        for tt in range(4 * qs, 4 * qs + 4):
                            for half in range(2):
                                oc = ocp.tile([P, HID // 2], BF16, tag="oc")
                                for hc in range(NHS // 2):  # 5 x 512-col chunks
                                    hs = half * (NHS // 2) + hc
                                    ps_f = psB.tile([P, 512], F32, tag="bp")
                                    nc.tensor.matmul(
                                        ps_f[:],
                                        OnT[:, 0, tt * 128 : (tt + 1) * 128],
                                        wo_sb[:, 0, hs * 512 : (hs + 1) * 512],
                                        start=True,
                                        stop=False,
                                    )
                                    nc.tensor.matmul(
                                        ps_f[:],
                                        OnT[:, 1, tt * 128 : (tt + 1) * 128],
                                        wo_sb[:, 1, hs * 512 : (hs + 1) * 512],
                                        start=False,
                                        stop=True,
                                    )
                                    # alternate copy engine so copies keep
                                    # pace with the matmul stream
                                    if hc % 2 == 0:
                                        nc.scalar.copy(
                                            oc[:, hc * 512 : (hc + 1) * 512], ps_f[:]
                                        )
                                    else:
                                        nc.vector.tensor_copy(
                                            oc[:, hc * 512 : (hc + 1) * 512], ps_f[:]
                                        )
                                nc.sync.dma_start(
                                    out[
                                        tt * 128 : (tt + 1) * 128,
                                        half * (HID // 2) : (half + 1) * (HID // 2),
                                    ],
                                    oc[:],
                                )

                    # interleave pe-rope slices, attention and w_o so DVE
                    # rope/divide latency hides behind PE matmul streams
                    q_pe_slice(0)
                    attn(0)
                    q_pe_slice(1)
                    wo_block(0)
                    attn(1)
                    q_pe_slice(2)
                    wo_block(1)
                    attn(2)
                    q_pe_slice(3)
                    wo_block(2)
                    attn(3)
                    wo_block(3)

    nc.compile()
    return nc


_NC_CACHE = None


def _get_nc():
    global _NC_CACHE
    if _NC_CACHE is None:
        _NC_CACHE = _build_nc()
    return _NC_CACHE


def _bf(x):
    return np.ascontiguousarray(x.astype(BF))


def _prep_in_maps(inputs):
    hidden = np.asarray(inputs["hidden_states"], dtype=np.float32)
    w_q_a = np.asarray(inputs["w_q_a"], dtype=np.float32)
    q_a_norm_w = np.asarray(inputs["q_a_norm_w"], dtype=np.float32)
    w_q_b = np.asarray(inputs["w_q_b"], dtype=np.float32)
    w_kv_a = np.asarray(inputs["w_kv_a"], dtype=np.float32)
    kv_a_norm_w = np.asarray(inputs["kv_a_norm_w"], dtype=np.float32)
    w_kv_b = np.asarray(inputs["w_kv_b"], dtype=np.float32)
    w_o = np.asarray(inputs["w_o"], dtype=np.float32)
    pos = np.asarray(inputs["positions"]).astype(np.float32)

    # rope tables, feature-major, evens/odds share the same row index
    inv_freq = _yarn_inv_freq()
    freqs = pos[:, None] * inv_freq[None, :]          # [T, 32]
    cosf = np.cos(freqs).T * COS_SIN_MSCALE           # [32, T]
    sinf = np.sin(freqs).T * COS_SIN_MSCALE
    cosf_b, sinf_b = _bf(cosf), _bf(sinf)
    cosl2 = np.concatenate([cosf_b, cosf_b], 0)       # duplicated halves
    sinl2 = np.concatenate([sinf_b, sinf_b], 0)

    # a-proj weights: [17 mtiles, 128p, 40k, 128c], pe cols de-interleaved
    wkva_pe = w_kv_a[:, KL:][:, PE_PERM]
    wa_full = np.concatenate(
        [w_q_a, w_kv_a[:, :KL], wkva_pe, np.zeros((HID, 64), np.float32)], axis=1
    )  # [5120, 2176]
    wa_l = _bf(wa_full.reshape(HCH, P, MT, P).transpose(2, 1, 0, 3))

    # fold RMSNorm gains + attention scale into b-proj weights
    wqb_s = w_q_b * q_a_norm_w[:, None] * ATTN_SCALE
    wkvb_s = w_kv_b * kv_a_norm_w[:, None]

    ones_b = _bf(np.ones((P, P), np.float32))
    tri_b = _bf(np.triu(np.ones((P, P), np.float32)))

    shared = {
        "wa": wa_l,
        "cosf": cosf_b,
        "sinf": sinf_b,
        "ones": ones_b,
        "tri": tri_b,
    }

    in_maps = []
    for c in range(NCORE):
        h0 = HPC * c
        # hidden slice, feature-major [128, 40, 256]
        hs = hidden[c * TLOC : (c + 1) * TLOC, :]
        hT_l = _bf(hs.T.reshape(HCH, P, TLOC).transpose(1, 0, 2))
        # w_q_b cols for this core's heads: [h0 nope | h1 nope | h0 pe | h1 pe]
        nope_cols, pe_cols = [], []
        for h in range(h0, h0 + HPC):
            blk = wqb_s[:, h * QK : (h + 1) * QK]
            nope_cols.append(blk[:, :NOPE])
            pe_cols.append(blk[:, NOPE:][:, PE_PERM])
        wqb_core = np.concatenate(nope_cols + pe_cols, axis=1)  # [1536, 384]
        wqb_l = _bf(wqb_core.reshape(QLC, P, HPC * QK).transpose(1, 0, 2))
        # w_kv_b cols: [h0 nope, h1 nope, h0 v, h1 v]
        nopes = [
            wkvb_s[:, h * (NOPE + VD) : h * (NOPE + VD) + NOPE]
            for h in range(h0, h0 + HPC)
        ]
        vs = [
            wkvb_s[:, h * (NOPE + VD) + NOPE : (h + 1) * (NOPE + VD)]
            for h in range(h0, h0 + HPC)
        ]
        wkvb_core = np.concatenate(nopes + vs, axis=1)  # [512, 512]
        wkvb_l = _bf(wkvb_core.reshape(KLC, P, 512).transpose(1, 0, 2))
        # w_o rows for this core's heads: [128, 2, 5120]
        wo_core = w_o[h0 * VD : (h0 + HPC) * VD, :]
        wo_l = _bf(wo_core.reshape(HPC, P, HID).transpose(1, 0, 2))

        m = dict(shared)
        m.update(
            {
                "hT": hT_l,
                "wqb": wqb_l,
                "wkvb": wkvb_l,
                "wo": wo_l,
                "cosl": np.ascontiguousarray(cosl2[:, c * TLOC : (c + 1) * TLOC]),
                "sinl": np.ascontiguousarray(sinl2[:, c * TLOC : (c + 1) * TLOC]),
            }
        )
        in_maps.append(m)
    return in_maps


def kernel(**inputs):
    global LAST_EXEC_NS
    nc = _get_nc()
    in_maps = _prep_in_maps(inputs)
    trace = os.environ.get("KERNEL_TRACE", "0") == "1"
    res = run_bass_kernel_spmd(
        nc, in_maps, core_ids=list(range(NCORE)), trace=trace
    )
    LAST_EXEC_NS = res.exec_time_ns
    out = res.results[0]["out"].astype(np.float32)
    for i in range(1, NCORE):
        out += res.results[i]["out"].astype(np.float32)
    return out



# revision 15
# speedup vs baseline: 1.0577x; 1.0234x over previous
"""DeepSeek MLA prefill on 8 TRN2 NeuronCores.

Sharding: tensor-parallel over heads (2 heads/core) for the b-projections,
attention and w_o (row-parallel -> host sums partials); sequence-parallel
a-projections (each core computes 256 tokens of q_a/kv_a/k_pe, normalizes,
ropes k_pe, then on-device AllGathers replicate the 2112x256 activations).
The kv-group a-proj runs first so its (small) gather and the whole kv
b-projection overlap the q-group a-proj and gather.

All activations that feed matmuls are kept feature-major ([d, T]) so no
on-device transposes are needed; v is produced token-major directly.
Matmuls run in bf16 with f32 PSUM accumulation (rel-err gate is ~2e-2).
"""

import math
import os

import ml_dtypes
import numpy as np

import concourse.bacc as bacc
import concourse.bass_isa as bass_isa
import concourse.mybir as mybir
import concourse.tile as tile
from concourse.bass_utils import run_bass_kernel_spmd

F32 = mybir.dt.float32
BF16 = mybir.dt.bfloat16
AF = mybir.ActivationFunctionType
ALU = mybir.AluOpType

# problem dims (hardcoded per contract)
T, HID, H = 2048, 5120, 16
QL, KL = 1536, 512
NOPE, ROPE, VD = 128, 64, 128
QK = NOPE + ROPE
EPS = 1e-6
NCORE = 8
HPC = H // NCORE          # heads per core = 2
TLOC = T // NCORE         # tokens per core = 256
P = 128
HCH = HID // P            # 40 hidden chunks
QLC = QL // P             # 12
KLC = KL // P             # 4
MT = QLC + KLC + 1        # 17 a-proj output tiles (12 q + 4 kv + 1 pe[64])
NKV = MT - QLC            # 5 kv-group tiles
NT = T // P               # 16 token tiles
NQS = 4                   # 512-wide q slices per head
NHS = HID // 512          # 10 output column slices

# yarn rope params
BASE, FACTOR = 10000.0, 40.0
BETA_FAST, BETA_SLOW, ORIG_MAX = 32.0, 1.0, 4096
MSCALE = 1.0
MSCALE_ALL_DIM = 1.0


def _yarn_get_mscale(scale, m):
    if scale <= 1.0:
        return 1.0
    return 0.1 * m * math.log(scale) + 1.0


def _yarn_inv_freq():
    pos_freqs = BASE ** (np.arange(0, ROPE, 2, dtype=np.float64) / ROPE)
    extra = 1.0 / pos_freqs
    inter = 1.0 / (FACTOR * pos_freqs)

    def corr_dim(n):
        return ROPE * math.log(ORIG_MAX / (n * 2 * math.pi)) / (2 * math.log(BASE))

    low = max(math.floor(corr_dim(BETA_FAST)), 0)
    high = min(math.ceil(corr_dim(BETA_SLOW)), ROPE - 1)
    ramp = np.clip(
        (np.arange(ROPE // 2, dtype=np.float64) - low) / max(high - low, 0.001),
        0.0,
        1.0,
    )
    mask = 1.0 - ramp
    return (inter * (1.0 - mask) + extra * mask).astype(np.float32)


COS_SIN_MSCALE = _yarn_get_mscale(FACTOR, MSCALE) / _yarn_get_mscale(
    FACTOR, MSCALE_ALL_DIM
)
_M = _yarn_get_mscale(FACTOR, MSCALE_ALL_DIM)
ATTN_SCALE = (QK ** -0.5) * _M * _M

BF = ml_dtypes.bfloat16
# de-interleave perm: even rope dims then odd rope dims
PE_PERM = np.concatenate([np.arange(0, ROPE, 2), np.arange(1, ROPE, 2)])

LAST_EXEC_NS = None


def _build_nc(single=False, reps=1):
    # single=True: no collective, 1 core — for cost-model timeline sims only
    nc = bacc.Bacc(
        "TRN2",
        target_bir_lowering=False,
        debug=False,
        num_devices=1 if single else NCORE,
    )

    hT = nc.dram_tensor("hT", [P, HCH, TLOC], BF16, kind="ExternalInput").ap()
    wa = nc.dram_tensor("wa", [MT, P, HCH, P], BF16, kind="ExternalInput").ap()
    wqb = nc.dram_tensor("wqb", [P, QLC, HPC * QK], BF16, kind="ExternalInput").ap()
    wkvb = nc.dram_tensor("wkvb", [P, KLC, 512], BF16, kind="ExternalInput").ap()
    wo = nc.dram_tensor("wo", [P, HPC, HID], BF16, kind="ExternalInput").ap()
    cosf = nc.dram_tensor("cosf", [ROPE // 2, T], BF16, kind="ExternalInput").ap()
    sinf = nc.dram_tensor("sinf", [ROPE // 2, T], BF16, kind="ExternalInput").ap()
    cosl = nc.dram_tensor("cosl", [ROPE, TLOC], BF16, kind="ExternalInput").ap()
    sinl = nc.dram_tensor("sinl", [ROPE, TLOC], BF16, kind="ExternalInput").ap()
    onesd = nc.dram_tensor("ones", [P, P], BF16, kind="ExternalInput").ap()
    trid = nc.dram_tensor("tri", [P, P], BF16, kind="ExternalInput").ap()
    out = nc.dram_tensor("out", [T, HID], BF16, kind="ExternalOutput").ap()

    locb = nc.dram_tensor("locb", [MT, P, TLOC], BF16).ap()
    gathkv = nc.dram_tensor(
        "gathkv", [NCORE, NKV, P, TLOC], BF16, addr_space="Shared"
    ).ap()
    gathq = nc.dram_tensor(
        "gathq", [NCORE, QLC, P, TLOC], BF16, addr_space="Shared"
    ).ap()

    with tile.TileContext(nc) as tc:
        with (
            tc.tile_pool(name="const", bufs=1) as cp,
            tc.tile_pool(name="persist", bufs=1) as pp,
            tc.tile_pool(name="ocp", bufs=3) as ocp,
        ):
            ones_sb = cp.tile([P, P], BF16, tag="ones")
            tri_sb = cp.tile([P, P], BF16, tag="tri")
            cosf_sb = cp.tile([ROPE // 2, T], BF16, tag="cosf")
            sinf_sb = cp.tile([ROPE // 2, T], BF16, tag="sinf")
            cosl_sb = cp.tile([ROPE, TLOC], BF16, tag="cosl")
            sinl_sb = cp.tile([ROPE, TLOC], BF16, tag="sinl")
            eps_sb = cp.tile([P, 1], F32, tag="eps")
            nc.vector.memset(eps_sb[:], EPS)

            # persistent attention operands (live across the phase transition)
            qTn = pp.tile([P, HPC, T], BF16, tag="qTn")
            # both heads' roped q_pe packed: rows [h0e h0o h1e h1o] x 32
            qTp = pp.tile([P, T], BF16, tag="qTp")
            kTn = pp.tile([P, HPC, T], BF16, tag="kTn")
            vtok = pp.tile([P, NT, HPC * VD], BF16, tag="vtok")
            OnT = pp.tile([P, HPC, T], BF16, tag="OnT")
            # k_pe duplicated into both 64-row halves so each head's score
            # matmul has lhsT/rhs at the same base partition (0 or 64)
            kpe = pp.tile([P, NCORE, TLOC], BF16, tag="kpe")
            wkvb_sb = pp.tile([P, KLC, 512], BF16, tag="wkvb")
            wqb_sb = pp.tile([P, QLC, HPC * QK], BF16, tag="wqb")
            wo_sb = pp.tile([P, HPC, HID], BF16, tag="wo")

            for _rep in range(reps):
                # ---------------- phase 1: local a-projections ----------------
                with (
                    tc.tile_pool(name="p1", bufs=1) as p1,
                    tc.tile_pool(name="wap", bufs=3) as wap,
                    tc.tile_pool(name="sqp", bufs=3) as sqp,
                    tc.tile_pool(name="ps1", bufs=3, space="PSUM") as ps1,
                    tc.tile_pool(name="pss", bufs=1, space="PSUM") as pss,
                ):
                    hT_sb = p1.tile([P, HCH, TLOC], BF16, tag="hT")
                    # only the first tiny chunk up front; the rest is
                    # interleaved with the first weight tile's chunks below
                    nc.sync.dma_start(hT_sb[:, 0:2, :], hT[:, 0:2, :])
                    araw = p1.tile([P, MT, TLOC], BF16, tag="araw")
                    anrm = p1.tile([P, MT, TLOC], BF16, tag="anrm")
                    # kv gather buffer lives only in phase 1 (kv_b consumes it)
                    kag = p1.tile([P, KLC, NCORE, TLOC], BF16, tag="kag")
                    kag_f = kag.rearrange("p m c t -> p m (c t)")
                    ssq = pss.tile([P, TLOC], F32, tag="ssq")
                    sskv = pss.tile([P, TLOC], F32, tag="sskv")

                    # kv-group mtiles first so their collective + the whole kv
                    # b-projection overlap the (3x bigger) q-group a-proj
                    for m in list(range(QLC, MT)) + list(range(QLC)):
                        wt = wap.tile([P, HCH, P], BF16, tag="wt")
                        if m == QLC:  # first mtile: pace-matched interleave of
                            # weight chunks and the rest of hT so the k-loop
                            # never waits long on either stream
                            nc.sync.dma_start(wt[:, 0:5, :], wa[m, :, 0:5, :])
                            nc.sync.dma_start(ones_sb[:], onesd)
                            nc.sync.dma_start(cosl_sb[:], cosl)
                            nc.sync.dma_start(sinl_sb[:], sinl)
                            for (w0, w1), (h0, h1) in [
                                ((5, 14), (2, 8)),
                                ((14, 24), (8, 16)),
                                ((24, 40), (16, 28)),
                                ((40, 40), (28, 40)),
                            ]:
                                if w1 > w0:
                                    nc.sync.dma_start(
                                        wt[:, w0:w1, :], wa[m, :, w0:w1, :]
                                    )
                                nc.sync.dma_start(hT_sb[:, h0:h1, :], hT[:, h0:h1, :])
                        else:
                            nc.sync.dma_start(wt[:], wa[m])
                            if m == QLC - 2:
                                # phase-2 q weights: late enough to not crowd
                                # the wa stream, early enough for phase 2
                                nc.sync.dma_start(wqb_sb[:], wqb)
                        ps = ps1.tile([P, TLOC], F32, tag="aps")
                        for k in range(HCH):
                            nc.tensor.matmul(
                                ps[:],
                                wt[:, k, :],
                                hT_sb[:, k, :],
                                start=(k == 0),
                                stop=(k == HCH - 1),
                            )
                        nc.scalar.copy(araw[:, m, :], ps[:])
                        if m < QLC + KLC:
                            sq = sqp.tile([P, TLOC], BF16, tag="sq")
                            nc.scalar.activation(sq[:], ps[:], AF.Square)
                            if m < QLC:
                                nc.tensor.matmul(
                                    ssq[:],
                                    ones_sb[:],
                                    sq[:],
                                    start=(m == 0),
                                    stop=(m == QLC - 1),
                                    skip_group_check=True,
                                )
                            else:
                                nc.tensor.matmul(
                                    sskv[:],
                                    ones_sb[:],
                                    sq[:],
                                    start=(m == QLC),
                                    stop=(m == QLC + KLC - 1),
                                    skip_group_check=True,
                                )

                        if m == MT - 1:
                            # kv group locally complete: normalize, rope, ship
                            rsq_k = p1.tile([P, TLOC], F32, tag="rsq_k")
                            tmpf2 = p1.tile([P, TLOC], F32, tag="tmpf2")
                            nc.scalar.activation(
                                tmpf2[:], sskv[:], AF.Sqrt,
                                bias=eps_sb[:], scale=1.0 / KL,
                            )
                            nc.vector.reciprocal(rsq_k[:], tmpf2[:])
                            for mm in range(QLC, QLC + KLC):
                                nc.vector.tensor_mul(
                                    anrm[:, mm, :], araw[:, mm, :], rsq_k[:]
                                )
                            # rope k_pe (rows 0:32 even, 32:64 odd of tile MT-1).
                            # Two-SBUF-input ops must share base partition, so
                            # cos/sin tables are duplicated across both halves.
                            t1 = p1.tile([ROPE, TLOC], BF16, tag="t1")
                            t2 = p1.tile([ROPE, TLOC], BF16, tag="t2")
                            xe = araw[0:32, MT - 1, :]
                            xo = araw[32:64, MT - 1, :]
                            nc.vector.tensor_mul(t1[0:32, :], xe, cosl_sb[0:32, :])
                            nc.vector.tensor_mul(t2[0:32, :], xo, sinl_sb[32:64, :])
                            nc.vector.tensor_sub(
                                anrm[0:32, MT - 1, :], t1[0:32, :], t2[0:32, :]
                            )
                            nc.vector.tensor_mul(t1[32:64, :], xo, cosl_sb[32:64, :])
                            nc.vector.tensor_mul(t2[32:64, :], xe, sinl_sb[0:32, :])
                            nc.vector.tensor_add(
                                anrm[32:64, MT - 1, :], t1[32:64, :], t2[32:64, :]
                            )
                            nc.vector.memset(anrm[64:128, MT - 1, :], 0.0)
                            nc.sync.dma_start(
                                locb[QLC:MT].rearrange("m p t -> p m t"),
                                anrm[:, QLC:MT, :],
                            )
                            if not single:
                                nc.gpsimd.collective_compute(
                                    "AllGather",
                                    ALU.bypass,
                                    replica_groups=[list(range(NCORE))],
                                    ins=[locb[QLC:MT].opt()],
                                    outs=[gathkv.opt()],
                                )
                            # kv gather-in + whole kv b-projection — overlaps
                            # the q-group a-proj matmuls still streaming on PE
                            nc.sync.dma_start(wkvb_sb[:], wkvb)
                            for mm in range(KLC):
                                nc.sync.dma_start(
                                    kag[:, mm],
                                    gathkv[:, mm].rearrange("c p t -> p c t"),
                                )
                            for half in range(2):
                                nc.sync.dma_start(
                                    kpe[half * ROPE : (half + 1) * ROPE],
                                    gathkv[:, NKV - 1, 0:ROPE, :].rearrange(
                                        "c p t -> p c t"
                                    ),
                                )
                            # k_nope^T per head: [128, T]
                            for hh in range(HPC):
                                for s in range(4):
                                    psb = ps1.tile([P, 512], F32, tag="bp")
                                    for k in range(KLC):
                                        nc.tensor.matmul(
                                            psb[:],
                                            wkvb_sb[:, k, hh * 128 : (hh + 1) * 128],
                                            kag_f[:, k, s * 512 : (s + 1) * 512],
                                            start=(k == 0),
                                            stop=(k == KLC - 1),
                                        )
                                    nc.scalar.copy(
                                        kTn[:, hh, s * 512 : (s + 1) * 512], psb[:]
                                    )
                            # v token-major: [t, 2*VD] per token tile
                            for tt in range(NT):
                                psb = ps1.tile([P, 512], F32, tag="bp")
                                for k in range(KLC):
                                    nc.tensor.matmul(
                                        psb[:, 0 : HPC * VD],
                                        kag[
                                            :, k, tt // 2,
                                            (tt % 2) * 128 : (tt % 2) * 128 + 128,
                                        ],
                                        wkvb_sb[:, k, 256:512],
                                        start=(k == 0),
                                        stop=(k == KLC - 1),
                                    )
                                nc.vector.tensor_copy(
                                    vtok[:, tt, :], psb[:, 0 : HPC * VD]
                                )

                    # q group: normalize + ship
                    rsq_q = p1.tile([P, TLOC], F32, tag="rsq_q")
                    tmpf = p1.tile([P, TLOC], F32, tag="tmpf")
                    nc.scalar.activation(
                        tmpf[:], ssq[:], AF.Sqrt, bias=eps_sb[:], scale=1.0 / QL
                    )
                    nc.vector.reciprocal(rsq_q[:], tmpf[:])
                    for m in range(QLC):
                        nc.vector.tensor_mul(anrm[:, m, :], araw[:, m, :], rsq_q[:])
                    nc.sync.dma_start(
                        locb[0:QLC].rearrange("m p t -> p m t"), anrm[:, 0:QLC, :]
                    )
                    if not single:
                        nc.gpsimd.collective_compute(
                            "AllGather",
                            ALU.bypass,
                            replica_groups=[list(range(NCORE))],
                            ins=[locb[0:QLC].opt()],
                            outs=[gathq.opt()],
                        )

                # ---------------- phase 2: q b-proj + attention + w_o ----------
                with (
                    tc.tile_pool(name="p2", bufs=1) as p2,
                    tc.tile_pool(name="ptp", bufs=2) as ptp,
                    tc.tile_pool(name="rcp", bufs=2) as rcp,
                    tc.tile_pool(name="dsp", bufs=2) as dsp,
                    tc.tile_pool(name="psB", bufs=3, space="PSUM") as psB,
                    tc.tile_pool(name="psA", bufs=3, space="PSUM") as psA,
                ):
                    qag = p2.tile([P, QLC, NCORE, TLOC], BF16, tag="qag")
                    for m in range(QLC):
                        nc.sync.dma_start(
                            qag[:, m], gathq[:, m].rearrange("c p t -> p c t")
                        )
                        if m == 5:
                            # rope tables needed once the pe slices start
                            nc.sync.dma_start(cosf_sb[:], cosf)
                            nc.sync.dma_start(sinf_sb[:], sinf)
                    nc.sync.dma_start(tri_sb[:], trid)
                    nc.sync.dma_start(wo_sb[:], wo)
                    qag_f = qag.rearrange("p m c t -> p m (c t)")

                    # q^T: nope [128, T] per head; both heads' pe packed M=128
                    # (wqb cols: [h0 nope | h1 nope | h0 pe | h1 pe])
                    for hh in range(HPC):
                        for s in range(4):
                            ps = psB.tile([P, 512], F32, tag="bp")
                            for k in range(QLC):
                                nc.tensor.matmul(
                                    ps[:],
                                    wqb_sb[:, k, hh * NOPE : (hh + 1) * NOPE],
                                    qag_f[:, k, s * 512 : (s + 1) * 512],
                                    start=(k == 0),
                                    stop=(k == QLC - 1),
                                )
                            nc.scalar.copy(qTn[:, hh, s * 512 : (s + 1) * 512], ps[:])

                    def q_pe_slice(s):
                        ps = psB.tile([P, 512], F32, tag="bp")
                        for k in range(QLC):
                            nc.tensor.matmul(
                                ps[:],
                                wqb_sb[:, k, HPC * NOPE : HPC * QK],
                                qag_f[:, k, s * 512 : (s + 1) * 512],
                                start=(k == 0),
                                stop=(k == QLC - 1),
                            )
                        # rope both heads' pe straight out of PSUM
                        # (PSUM x SBUF ops are exempt from the equal-base rule)
                        sl = slice(s * 512, (s + 1) * 512)
                        cs, sn = cosf_sb[:, sl], sinf_sb[:, sl]
                        rt = p2.tile([P, 512], BF16, tag="rt")
                        for hh in range(HPC):
                            b = hh * ROPE
                            xe, xo = ps[b : b + 32, :], ps[b + 32 : b + 64, :]
                            nc.vector.tensor_mul(qTp[b : b + 32, sl], xe, cs)
                            nc.vector.tensor_mul(rt[b : b + 32, :], xo, sn)
                            nc.vector.tensor_sub(
                                qTp[b : b + 32, sl],
                                qTp[b : b + 32, sl],
                                rt[b : b + 32, :],
                            )
                            nc.vector.tensor_mul(qTp[b + 32 : b + 64, sl], xo, cs)
                            nc.vector.tensor_mul(rt[b + 32 : b + 64, :], xe, sn)
                            nc.vector.tensor_add(
                                qTp[b + 32 : b + 64, sl],
                                qTp[b + 32 : b + 64, sl],
                                rt[b + 32 : b + 64, :],
                            )

                    # attention (S^T layout, no max-subtraction); softmax
                    # denominator accumulated on DVE + Pool partition-reduce
                    # instead of PE ones-matmuls
                    def attn(qs):
                        for hh in range(HPC):
                            nk = 4 * qs + 4
                            PT = ptp.tile([P, NT, 512], BF16, tag="PT")
                            # two partial accumulators: even k-tiles on DVE,
                            # odd on Pool — splits the add load across engines
                            dsum = dsp.tile([P, 512], F32, tag="dsum")
                            dsumB = dsp.tile([P, 512], F32, tag="dsumB")
                            for kt in range(nk):
                                # columns 0..r*128 are fully causal-masked:
                                # skip them in every matmul of this k-tile
                                r = kt - 4 * qs
                                c0 = max(r, 0) * 128
                                ps_s = psA.tile([P, 512], F32, tag="ps_s")
                                nc.tensor.matmul(
                                    ps_s[:, c0:512],
                                    kTn[:, hh, kt * 128 : (kt + 1) * 128],
                                    qTn[:, hh, qs * 512 + c0 : (qs + 1) * 512],
                                    start=True,
                                    stop=False,
                                )
                                hb = hh * ROPE
                                nc.tensor.matmul(
                                    ps_s[:, c0:512],
                                    kpe[
                                        hb : hb + ROPE, kt // 2,
                                        (kt % 2) * 128 : (kt % 2) * 128 + 128,
                                    ],
                                    qTp[hb : hb + ROPE, qs * 512 + c0 : (qs + 1) * 512],
                                    start=False,
                                    stop=True,
                                )
                                if c0 > 0:
                                    nc.vector.memset(PT[:, kt, 0:c0], 0.0)
                                nc.scalar.activation(
                                    PT[:, kt, c0:512], ps_s[:, c0:512], AF.Exp
                                )
                                if 0 <= r <= 3:
                                    nc.vector.tensor_mul(
                                        PT[:, kt, r * 128 : (r + 1) * 128],
                                        PT[:, kt, r * 128 : (r + 1) * 128],
                                        tri_sb[:],
                                    )
                                eng = nc.vector if kt % 2 == 0 else nc.gpsimd
                                acc = dsum if kt % 2 == 0 else dsumB
                                if kt < 2:
                                    eng.tensor_copy(acc[:], PT[:, kt, :])
                                else:
                                    eng.tensor_add(acc[:], acc[:], PT[:, kt, :])
                            ps_o = psB.tile([P, 512], F32, tag="bp")
                            for kt in range(nk):
                                c0 = max(kt - 4 * qs, 0) * 128
                                nc.tensor.matmul(
                                    ps_o[:, c0:512],
                                    vtok[:, kt, hh * VD : (hh + 1) * VD],
                                    PT[:, kt, c0:512],
                                    start=(kt == 0),
                                    stop=(kt == nk - 1),
                                )
                            dred = dsp.tile([P, 512], F32, tag="dred")
                            nc.vector.tensor_add(dsum[:], dsum[:], dsumB[:])
                            nc.gpsimd.partition_all_reduce(
                                dred[:], dsum[:], channels=128,
                                reduce_op=bass_isa.ReduceOp.add,
                            )
                            rec = rcp.tile([P, 512], F32, tag="rec")
                            nc.vector.reciprocal(rec[:], dred[:])
                            nc.vector.tensor_mul(
                                OnT[:, hh, qs * 512 : (qs + 1) * 512], ps_o[:], rec[:]
                            )

                    def wo_block(qs):
                        for tt in range(4 * qs, 4 * qs + 4):
                            for half in range(2):
                                oc = ocp.tile([P, HID // 2], BF16, tag="oc")
                                for hc in range(NHS // 2):  # 5 x 512-col chunks
                                    hs = half * (NHS // 2) + hc
                                    ps_f = psB.tile([P, 512], F32, tag="bp")
                                    nc.tensor.matmul(
                                        ps_f[:],
                                        OnT[:, 0, tt * 128 : (tt + 1) * 128],
                                        wo_sb[:, 0, hs * 512 : (hs + 1) * 512],
                                        start=True,
                                        stop=False,
                                    )
                                    nc.tensor.matmul(
                                        ps_f[:],
                                        OnT[:, 1, tt * 128 : (tt + 1) * 128],
                                        wo_sb[:, 1, hs * 512 : (hs + 1) * 512],
                                        start=False,
                                        stop=True,
                                    )
                                    # alternate copy engine so copies keep
                                    # pace with the matmul stream
                                    if hc % 2 == 0:
                                        nc.scalar.copy(
                                            oc[:, hc * 512 : (hc + 1) * 512], ps_f[:]
                                        )
                                    else:
                                        nc.vector.tensor_copy(
                                            oc[:, hc * 512 : (hc + 1) * 512], ps_f[:]
                                        )
                                nc.sync.dma_start(
                                    out[
                                        tt * 128 : (tt + 1) * 128,
                                        half * (HID // 2) : (half + 1) * (HID // 2),
                                    ],
                                    oc[:],
                                )

                    # interleave pe-rope slices, attention and w_o so DVE
                    # rope/divide latency hides behind PE matmul streams
                    q_pe_slice(0)
                    attn(0)
                    q_pe_slice(1)
                    wo_block(0)
                    attn(1)
                    q_pe_slice(2)
                    wo_block(1)
                    attn(2)
                    q_pe_slice(3)
                    wo_block(2)
                    attn(3)
                    wo_block(3)

    nc.compile()
    return nc


_NC_CACHE = None


def _get_nc():
    global _NC_CACHE
    if _NC_CACHE is None:
        _NC_CACHE = _build_nc()
    return _NC_CACHE


def _bf(x):
    return np.ascontiguousarray(x.astype(BF))


def _prep_in_maps(inputs):
    hidden = np.asarray(inputs["hidden_states"], dtype=np.float32)
    w_q_a = np.asarray(inputs["w_q_a"], dtype=np.float32)
    q_a_norm_w = np.asarray(inputs["q_a_norm_w"], dtype=np.float32)
    w_q_b = np.asarray(inputs["w_q_b"], dtype=np.float32)
    w_kv_a = np.asarray(inputs["w_kv_a"], dtype=np.float32)
    kv_a_norm_w = np.asarray(inputs["kv_a_norm_w"], dtype=np.float32)
    w_kv_b = np.asarray(inputs["w_kv_b"], dtype=np.float32)
    w_o = np.asarray(inputs["w_o"], dtype=np.float32)
    pos = np.asarray(inputs["positions"]).astype(np.float32)

    # rope tables, feature-major, evens/odds share the same row index
    inv_freq = _yarn_inv_freq()
    freqs = pos[:, None] * inv_freq[None, :]          # [T, 32]
    cosf = np.cos(freqs).T * COS_SIN_MSCALE           # [32, T]
    sinf = np.sin(freqs).T * COS_SIN_MSCALE
    cosf_b, sinf_b = _bf(cosf), _bf(sinf)
    cosl2 = np.concatenate([cosf_b, cosf_b], 0)       # duplicated halves
    sinl2 = np.concatenate([sinf_b, sinf_b], 0)

    # a-proj weights: [17 mtiles, 128p, 40k, 128c], pe cols de-interleaved
    wkva_pe = w_kv_a[:, KL:][:, PE_PERM]
    wa_full = np.concatenate(
        [w_q_a, w_kv_a[:, :KL], wkva_pe, np.zeros((HID, 64), np.float32)], axis=1
    )  # [5120, 2176]
    wa_l = _bf(wa_full.reshape(HCH, P, MT, P).transpose(2, 1, 0, 3))

    # fold RMSNorm gains + attention scale into b-proj weights
    wqb_s = w_q_b * q_a_norm_w[:, None] * ATTN_SCALE
    wkvb_s = w_kv_b * kv_a_norm_w[:, None]

    ones_b = _bf(np.ones((P, P), np.float32))
    tri_b = _bf(np.triu(np.ones((P, P), np.float32)))

    shared = {
        "wa": wa_l,
        "cosf": cosf_b,
        "sinf": sinf_b,
        "ones": ones_b,
        "tri": tri_b,
    }

    in_maps = []
    for c in range(NCORE):
        h0 = HPC * c
        # hidden slice, feature-major [128, 40, 256]
        hs = hidden[c * TLOC : (c + 1) * TLOC, :]
        hT_l = _bf(hs.T.reshape(HCH, P, TLOC).transpose(1, 0, 2))
        # w_q_b cols for this core's heads: [h0 nope | h1 nope | h0 pe | h1 pe]
        nope_cols, pe_cols = [], []
        for h in range(h0, h0 + HPC):
            blk = wqb_s[:, h * QK : (h + 1) * QK]
            nope_cols.append(blk[:, :NOPE])
            pe_cols.append(blk[:, NOPE:][:, PE_PERM])
        wqb_core = np.concatenate(nope_cols + pe_cols, axis=1)  # [1536, 384]
        wqb_l = _bf(wqb_core.reshape(QLC, P, HPC * QK).transpose(1, 0, 2))
        # w_kv_b cols: [h0 nope, h1 nope, h0 v, h1 v]
        nopes = [
            wkvb_s[:, h * (NOPE + VD) : h * (NOPE + VD) + NOPE]
            for h in range(h0, h0 + HPC)
        ]
        vs = [
            wkvb_s[:, h * (NOPE + VD) + NOPE : (h + 1) * (NOPE + VD)]
            for h in range(h0, h0 + HPC)
        ]
        wkvb_core = np.concatenate(nopes + vs, axis=1)  # [512, 512]
        wkvb_l = _bf(wkvb_core.reshape(KLC, P, 512).transpose(1, 0, 2))
        # w_o rows for this core's heads: [128, 2, 5120]
        wo_core = w_o[h0 * VD : (h0 + HPC) * VD, :]
        wo_l = _bf(wo_core.reshape(HPC, P, HID).transpose(1, 0, 2))

        m = dict(shared)
        m.update(
            {
                "hT": hT_l,
                "wqb": wqb_l,
                "wkvb": wkvb_l,
                "wo": wo_l,
                "cosl": np.ascontiguousarray(cosl2[:, c * TLOC : (c + 1) * TLOC]),
                "sinl": np.ascontiguousarray(sinl2[:, c * TLOC : (c + 1) * TLOC]),
            }
        )
        in_maps.append(m)
    return in_maps


def kernel(**inputs):
    global LAST_EXEC_NS
    nc = _get_nc()
    in_maps = _prep_in_maps(inputs)
    trace = os.environ.get("KERNEL_TRACE", "0") == "1"
    res = run_bass_kernel_spmd(
        nc, in_maps, core_ids=list(range(NCORE)), trace=trace
    )
    LAST_EXEC_NS = res.exec_time_ns
    out = res.results[0]["out"].astype(np.float32)
    for i in range(1, NCORE):
        out += res.results[i]["out"].astype(np.float32)
    return out



# revision 28
# speedup vs baseline: 1.0675x; 1.0094x over previous
"""DeepSeek MLA prefill on 8 TRN2 NeuronCores.

Sharding: tensor-parallel over heads (2 heads/core) for the b-projections,
attention and w_o (row-parallel -> host sums partials); sequence-parallel
a-projections (each core computes 256 tokens of q_a/kv_a/k_pe, normalizes,
ropes k_pe, then on-device AllGathers replicate the 2112x256 activations).
The kv-group a-proj runs first so its (small) gather and the whole kv
b-projection overlap the q-group a-proj and gather.

All activations that feed matmuls are kept feature-major ([d, T]) so no
on-device transposes are needed; v is produced token-major directly.
Matmuls run in bf16 with f32 PSUM accumulation (rel-err gate is ~2e-2).
"""

import math
import os

import ml_dtypes
import numpy as np

import concourse.bacc as bacc
import concourse.bass_isa as bass_isa
import concourse.mybir as mybir
import concourse.tile as tile
from concourse.bass_utils import run_bass_kernel_spmd

F32 = mybir.dt.float32
BF16 = mybir.dt.bfloat16
AF = mybir.ActivationFunctionType
ALU = mybir.AluOpType

# problem dims (hardcoded per contract)
T, HID, H = 2048, 5120, 16
QL, KL = 1536, 512
NOPE, ROPE, VD = 128, 64, 128
QK = NOPE + ROPE
EPS = 1e-6
NCORE = 8
HPC = H // NCORE          # heads per core = 2
TLOC = T // NCORE         # tokens per core = 256
P = 128
HCH = HID // P            # 40 hidden chunks
QLC = QL // P             # 12
KLC = KL // P             # 4
MT = QLC + KLC + 1        # 17 a-proj output tiles (12 q + 4 kv + 1 pe[64])
NKV = MT - QLC            # 5 kv-group tiles
NT = T // P               # 16 token tiles
NQS = 4                   # 512-wide q slices per head
NHS = HID // 512          # 10 output column slices

# yarn rope params
BASE, FACTOR = 10000.0, 40.0
BETA_FAST, BETA_SLOW, ORIG_MAX = 32.0, 1.0, 4096
MSCALE = 1.0
MSCALE_ALL_DIM = 1.0


def _yarn_get_mscale(scale, m):
    if scale <= 1.0:
        return 1.0
    return 0.1 * m * math.log(scale) + 1.0


def _yarn_inv_freq():
    pos_freqs = BASE ** (np.arange(0, ROPE, 2, dtype=np.float64) / ROPE)
    extra = 1.0 / pos_freqs
    inter = 1.0 / (FACTOR * pos_freqs)

    def corr_dim(n):
        return ROPE * math.log(ORIG_MAX / (n * 2 * math.pi)) / (2 * math.log(BASE))

    low = max(math.floor(corr_dim(BETA_FAST)), 0)
    high = min(math.ceil(corr_dim(BETA_SLOW)), ROPE - 1)
    ramp = np.clip(
        (np.arange(ROPE // 2, dtype=np.float64) - low) / max(high - low, 0.001),
        0.0,
        1.0,
    )
    mask = 1.0 - ramp
    return (inter * (1.0 - mask) + extra * mask).astype(np.float32)


COS_SIN_MSCALE = _yarn_get_mscale(FACTOR, MSCALE) / _yarn_get_mscale(
    FACTOR, MSCALE_ALL_DIM
)
_M = _yarn_get_mscale(FACTOR, MSCALE_ALL_DIM)
ATTN_SCALE = (QK ** -0.5) * _M * _M

BF = ml_dtypes.bfloat16
# de-interleave perm: even rope dims then odd rope dims
PE_PERM = np.concatenate([np.arange(0, ROPE, 2), np.arange(1, ROPE, 2)])

LAST_EXEC_NS = None


def _build_nc(single=False, reps=1):
    # single=True: no collective, 1 core — for cost-model timeline sims only
    nc = bacc.Bacc(
        "TRN2",
        target_bir_lowering=False,
        debug=False,
        num_devices=1 if single else NCORE,
    )

    hT = nc.dram_tensor("hT", [P, HCH, TLOC], BF16, kind="ExternalInput").ap()
    wa = nc.dram_tensor("wa", [MT, P, HCH, P], BF16, kind="ExternalInput").ap()
    wqb = nc.dram_tensor("wqb", [P, QLC, HPC * QK], BF16, kind="ExternalInput").ap()
    wkvb = nc.dram_tensor("wkvb", [P, KLC, 512], BF16, kind="ExternalInput").ap()
    wo = nc.dram_tensor("wo", [P, HPC, HID], BF16, kind="ExternalInput").ap()
    cosf = nc.dram_tensor("cosf", [ROPE // 2, T], BF16, kind="ExternalInput").ap()
    sinf = nc.dram_tensor("sinf", [ROPE // 2, T], BF16, kind="ExternalInput").ap()
    cosl = nc.dram_tensor("cosl", [ROPE, TLOC], BF16, kind="ExternalInput").ap()
    sinl = nc.dram_tensor("sinl", [ROPE, TLOC], BF16, kind="ExternalInput").ap()
    onesd = nc.dram_tensor("ones", [P, P], BF16, kind="ExternalInput").ap()
    trid = nc.dram_tensor("tri", [P, P], BF16, kind="ExternalInput").ap()
    out = nc.dram_tensor("out", [T, HID], BF16, kind="ExternalOutput").ap()

    locb = nc.dram_tensor("locb", [MT, P, TLOC], BF16).ap()
    locr = nc.dram_tensor("locr", [P, TLOC], F32).ap()
    gathkv = nc.dram_tensor(
        "gathkv", [NCORE, NKV, P, TLOC], BF16, addr_space="Shared"
    ).ap()
    # q latents gathered RAW (pre-norm) in 2-mtile chunks as the a-proj
    # produces them; the rsq row-norms gather separately and are applied
    # after the q b-projection (scaling commutes through the matmul)
    gathq = [
        nc.dram_tensor(f"gathq{i}", [NCORE, 2, P, TLOC], BF16, addr_space="Shared").ap()
        for i in range(QLC // 2)
    ]
    gathr = nc.dram_tensor("gathr", [NCORE, P, TLOC], F32, addr_space="Shared").ap()

    with tile.TileContext(nc) as tc:
        with (
            tc.tile_pool(name="const", bufs=1) as cp,
            tc.tile_pool(name="persist", bufs=1) as pp,
            tc.tile_pool(name="ocp", bufs=3) as ocp,
        ):
            ones_sb = cp.tile([P, P], BF16, tag="ones")
            tri_sb = cp.tile([P, P], BF16, tag="tri")
            cosf_sb = cp.tile([ROPE // 2, T], BF16, tag="cosf")
            sinf_sb = cp.tile([ROPE // 2, T], BF16, tag="sinf")
            cosl_sb = cp.tile([ROPE, TLOC], BF16, tag="cosl")
            sinl_sb = cp.tile([ROPE, TLOC], BF16, tag="sinl")
            eps_sb = cp.tile([P, 1], F32, tag="eps")
            nc.vector.memset(eps_sb[:], EPS)

            # persistent attention operands (live across the phase transition)
            qTn = pp.tile([P, HPC, T], BF16, tag="qTn")
            # both heads' roped q_pe packed: rows [h0e h0o h1e h1o] x 32
            qTp = pp.tile([P, T], BF16, tag="qTp")
            kTn = pp.tile([P, HPC, T], BF16, tag="kTn")
            vtok = pp.tile([P, NT, HPC * VD], BF16, tag="vtok")
            OnT = pp.tile([P, HPC, T], BF16, tag="OnT")
            # k_pe duplicated into both 64-row halves so each head's score
            # matmul has lhsT/rhs at the same base partition (0 or 64)
            kpe = pp.tile([P, NCORE, TLOC], BF16, tag="kpe")
            wkvb_sb = pp.tile([P, KLC, 512], BF16, tag="wkvb")
            wqb_sb = pp.tile([P, QLC, HPC * QK], BF16, tag="wqb")
            # raw q latents (all cores) + per-token rsq, filled during phase 1
            qag = pp.tile([P, QLC, NCORE, TLOC], BF16, tag="qag")
            qag_f = qag.rearrange("p m c t -> p m (c t)")
            rsqf = pp.tile([P, NCORE, TLOC], F32, tag="rsqf")
            rsqf_f = rsqf.rearrange("p c t -> p (c t)")

            for _rep in range(reps):
                # ---------------- phase 1: local a-projections ----------------
                with (
                    tc.tile_pool(name="p1", bufs=1) as p1,
                    tc.tile_pool(name="wap", bufs=2) as wap,
                    tc.tile_pool(name="sqp", bufs=3) as sqp,
                    tc.tile_pool(name="ps1", bufs=3, space="PSUM") as ps1,
                    tc.tile_pool(name="pss", bufs=1, space="PSUM") as pss,
                ):
                    hT_sb = p1.tile([P, HCH, TLOC], BF16, tag="hT")
                    # only the first tiny chunk up front; the rest is
                    # interleaved with the first weight tile's chunks below
                    nc.sync.dma_start(hT_sb[:, 0:2, :], hT[:, 0:2, :])
                    araw = p1.tile([P, MT, TLOC], BF16, tag="araw")
                    anrm = p1.tile([P, NKV, TLOC], BF16, tag="anrm")
                    # kv gather buffer lives only in phase 1 (kv_b consumes it)
                    kag = p1.tile([P, KLC, NCORE, TLOC], BF16, tag="kag")
                    kag_f = kag.rearrange("p m c t -> p m (c t)")
                    ssq = pss.tile([P, TLOC], F32, tag="ssq")
                    sskv = pss.tile([P, TLOC], F32, tag="sskv")

                    # kv-group mtiles first so their collective + the whole kv
                    # b-projection overlap the (3x bigger) q-group a-proj
                    for m in list(range(QLC, MT)) + list(range(QLC)):
                        wt = wap.tile([P, HCH, P], BF16, tag="wt")
                        if m == QLC:  # first mtile: pace-matched interleave of
                            # weight chunks and the rest of hT so the k-loop
                            # never waits long on either stream
                            nc.sync.dma_start(wt[:, 0:5, :], wa[m, :, 0:5, :])
                            nc.sync.dma_start(ones_sb[:], onesd)
                            nc.sync.dma_start(cosl_sb[:], cosl)
                            nc.sync.dma_start(sinl_sb[:], sinl)
                            for (w0, w1), (h0, h1) in [
                                ((5, 14), (2, 8)),
                                ((14, 24), (8, 16)),
                                ((24, 40), (16, 28)),
                                ((40, 40), (28, 40)),
                            ]:
                                if w1 > w0:
                                    nc.sync.dma_start(
                                        wt[:, w0:w1, :], wa[m, :, w0:w1, :]
                                    )
                                nc.sync.dma_start(hT_sb[:, h0:h1, :], hT[:, h0:h1, :])
                        else:
                            nc.sync.dma_start(wt[:], wa[m])
                            if m == QLC - 2:
                                # phase-2 q weights: late enough to not crowd
                                # the wa stream, early enough for phase 2
                                nc.sync.dma_start(wqb_sb[:], wqb)
                        ps = ps1.tile([P, TLOC], F32, tag="aps")
                        for k in range(HCH):
                            nc.tensor.matmul(
                                ps[:],
                                wt[:, k, :],
                                hT_sb[:, k, :],
                                start=(k == 0),
                                stop=(k == HCH - 1),
                            )
                        nc.scalar.copy(araw[:, m, :], ps[:])
                        if m < QLC + KLC:
                            sq = sqp.tile([P, TLOC], BF16, tag="sq")
                            nc.scalar.activation(sq[:], ps[:], AF.Square)
                            if m < QLC:
                                nc.tensor.matmul(
                                    ssq[:],
                                    ones_sb[:],
                                    sq[:],
                                    start=(m == 0),
                                    stop=(m == QLC - 1),
                                    skip_group_check=True,
                                )
                            else:
                                nc.tensor.matmul(
                                    sskv[:],
                                    ones_sb[:],
                                    sq[:],
                                    start=(m == QLC),
                                    stop=(m == QLC + KLC - 1),
                                    skip_group_check=True,
                                )

                        if m < QLC and m % 2 == 1:
                            # ship this RAW q-latent pair: write, gather, and
                            # pull into SBUF while later mtiles still compute
                            cch = m // 2
                            nc.sync.dma_start(
                                locb[m - 1 : m + 1].rearrange("m p t -> p m t"),
                                araw[:, m - 1 : m + 1, :],
                            )
                            if not single:
                                nc.gpsimd.collective_compute(
                                    "AllGather",
                                    ALU.bypass,
                                    replica_groups=[list(range(NCORE))],
                                    ins=[locb[m - 1 : m + 1].opt()],
                                    outs=[gathq[cch].opt()],
                                )
                            for j in range(2):
                                nc.sync.dma_start(
                                    qag[:, m - 1 + j],
                                    gathq[cch][:, j].rearrange("c p t -> p c t"),
                                )

                        if m == MT - 1:
                            # kv group locally complete: normalize, rope, ship
                            rsq_k = p1.tile([P, TLOC], F32, tag="rsq_k")
                            tmpf2 = p1.tile([P, TLOC], F32, tag="tmpf2")
                            nc.scalar.activation(
                                tmpf2[:], sskv[:], AF.Sqrt,
                                bias=eps_sb[:], scale=1.0 / KL,
                            )
                            nc.vector.reciprocal(rsq_k[:], tmpf2[:])
                            for mm in range(QLC, QLC + KLC):
                                nc.vector.tensor_mul(
                                    anrm[:, mm - QLC, :], araw[:, mm, :], rsq_k[:]
                                )
                            # rope k_pe (rows 0:32 even, 32:64 odd of tile MT-1).
                            # Two-SBUF-input ops must share base partition, so
                            # cos/sin tables are duplicated across both halves.
                            t1 = p1.tile([ROPE, TLOC], BF16, tag="t1")
                            t2 = p1.tile([ROPE, TLOC], BF16, tag="t2")
                            xe = araw[0:32, MT - 1, :]
                            xo = araw[32:64, MT - 1, :]
                            nc.vector.tensor_mul(t1[0:32, :], xe, cosl_sb[0:32, :])
                            nc.vector.tensor_mul(t2[0:32, :], xo, sinl_sb[32:64, :])
                            nc.vector.tensor_sub(
                                anrm[0:32, NKV - 1, :], t1[0:32, :], t2[0:32, :]
                            )
                            nc.vector.tensor_mul(t1[32:64, :], xo, cosl_sb[32:64, :])
                            nc.vector.tensor_mul(t2[32:64, :], xe, sinl_sb[0:32, :])
                            nc.vector.tensor_add(
                                anrm[32:64, NKV - 1, :], t1[32:64, :], t2[32:64, :]
                            )
                            nc.vector.memset(anrm[64:128, NKV - 1, :], 0.0)
                            nc.sync.dma_start(
                                locb[QLC:MT].rearrange("m p t -> p m t"),
                                anrm[:, 0:NKV, :],
                            )
                            if not single:
                                nc.gpsimd.collective_compute(
                                    "AllGather",
                                    ALU.bypass,
                                    replica_groups=[list(range(NCORE))],
                                    ins=[locb[QLC:MT].opt()],
                                    outs=[gathkv.opt()],
                                )
                            # kv gather-in + whole kv b-projection — overlaps
                            # the q-group a-proj matmuls still streaming on PE
                            nc.sync.dma_start(wkvb_sb[:], wkvb)
                            for mm in range(KLC):
                                nc.sync.dma_start(
                                    kag[:, mm],
                                    gathkv[:, mm].rearrange("c p t -> p c t"),
                                )
                            for half in range(2):
                                nc.sync.dma_start(
                                    kpe[half * ROPE : (half + 1) * ROPE],
                                    gathkv[:, NKV - 1, 0:ROPE, :].rearrange(
                                        "c p t -> p c t"
                                    ),
                                )
                            # k_nope^T per head: [128, T]
                            for hh in range(HPC):
                                for s in range(4):
                                    psb = ps1.tile([P, 512], F32, tag="bp")
                                    for k in range(KLC):
                                        nc.tensor.matmul(
                                            psb[:],
                                            wkvb_sb[:, k, hh * 128 : (hh + 1) * 128],
                                            kag_f[:, k, s * 512 : (s + 1) * 512],
                                            start=(k == 0),
                                            stop=(k == KLC - 1),
                                        )
                                    nc.scalar.copy(
                                        kTn[:, hh, s * 512 : (s + 1) * 512], psb[:]
                                    )
                            # v token-major: [t, 2*VD] per token tile
                            for tt in range(NT):
                                psb = ps1.tile([P, 512], F32, tag="bp")
                                for k in range(KLC):
                                    nc.tensor.matmul(
                                        psb[:, 0 : HPC * VD],
                                        kag[
                                            :, k, tt // 2,
                                            (tt % 2) * 128 : (tt % 2) * 128 + 128,
                                        ],
                                        wkvb_sb[:, k, 256:512],
                                        start=(k == 0),
                                        stop=(k == KLC - 1),
                                    )
                                nc.vector.tensor_copy(
                                    vtok[:, tt, :], psb[:, 0 : HPC * VD]
                                )

                    # q row-norms: gather the per-token rsq instead of the
                    # normalized activations (applied post-b-proj in phase 2)
                    rsq_q = p1.tile([P, TLOC], F32, tag="rsq_q")
                    tmpf = p1.tile([P, TLOC], F32, tag="tmpf")
                    nc.scalar.activation(
                        tmpf[:], ssq[:], AF.Sqrt, bias=eps_sb[:], scale=1.0 / QL
                    )
                    nc.vector.reciprocal(rsq_q[:], tmpf[:])
                    nc.sync.dma_start(locr, rsq_q[:])
                    if not single:
                        nc.gpsimd.collective_compute(
                            "AllGather",
                            ALU.bypass,
                            replica_groups=[list(range(NCORE))],
                            ins=[locr.opt()],
                            outs=[gathr.opt()],
                        )
                    nc.sync.dma_start(rsqf[:], gathr.rearrange("c p t -> p c t"))

                # ---------------- phase 2: q b-proj + attention + w_o ----------
                with (
                    tc.tile_pool(name="p2", bufs=1) as p2,
                    tc.tile_pool(name="ptp", bufs=2) as ptp,
                    tc.tile_pool(name="rcp", bufs=2) as rcp,
                    tc.tile_pool(name="dsp", bufs=2) as dsp,
                    tc.tile_pool(name="psB", bufs=3, space="PSUM") as psB,
                    tc.tile_pool(name="psA", bufs=3, space="PSUM") as psA,
                ):
                    # w_o weights only live in phase 2
                    wo_sb = p2.tile([P, HPC, HID], BF16, tag="wo")
                    nc.sync.dma_start(cosf_sb[:], cosf)
                    nc.sync.dma_start(sinf_sb[:], sinf)
                    nc.sync.dma_start(tri_sb[:], trid)
                    nc.sync.dma_start(wo_sb[:], wo)

                    # q^T: nope [128, T] per head; both heads' pe packed M=128
                    # (wqb cols: [h0 nope | h1 nope | h0 pe | h1 pe]);
                    # rsq row-norm folded into the PSUM->SBUF move
                    for hh in range(HPC):
                        for s in range(4):
                            ps = psB.tile([P, 512], F32, tag="bp")
                            for k in range(QLC):
                                nc.tensor.matmul(
                                    ps[:],
                                    wqb_sb[:, k, hh * NOPE : (hh + 1) * NOPE],
                                    qag_f[:, k, s * 512 : (s + 1) * 512],
                                    start=(k == 0),
                                    stop=(k == QLC - 1),
                                )
                            nc.vector.tensor_mul(
                                qTn[:, hh, s * 512 : (s + 1) * 512],
                                ps[:],
                                rsqf_f[:, s * 512 : (s + 1) * 512],
                            )

                    def q_pe_slice(s):
                        ps = psB.tile([P, 512], F32, tag="bp")
                        for k in range(QLC):
                            nc.tensor.matmul(
                                ps[:],
                                wqb_sb[:, k, HPC * NOPE : HPC * QK],
                                qag_f[:, k, s * 512 : (s + 1) * 512],
                                start=(k == 0),
                                stop=(k == QLC - 1),
                            )
                        # rope both heads' pe straight out of PSUM
                        # (PSUM x SBUF ops are exempt from the equal-base rule)
                        sl = slice(s * 512, (s + 1) * 512)
                        cs, sn = cosf_sb[:, sl], sinf_sb[:, sl]
                        rt = p2.tile([P, 512], BF16, tag="rt")
                        for hh in range(HPC):
                            b = hh * ROPE
                            xe, xo = ps[b : b + 32, :], ps[b + 32 : b + 64, :]
                            nc.vector.tensor_mul(qTp[b : b + 32, sl], xe, cs)
                            nc.vector.tensor_mul(rt[b : b + 32, :], xo, sn)
                            nc.vector.tensor_sub(
                                qTp[b : b + 32, sl],
                                qTp[b : b + 32, sl],
                                rt[b : b + 32, :],
                            )
                            nc.vector.tensor_mul(qTp[b + 32 : b + 64, sl], xo, cs)
                            nc.vector.tensor_mul(rt[b + 32 : b + 64, :], xe, sn)
                            nc.vector.tensor_add(
                                qTp[b + 32 : b + 64, sl],
                                qTp[b + 32 : b + 64, sl],
                                rt[b + 32 : b + 64, :],
                            )
                        # apply the q rsq row-norm (commutes with rope)
                        nc.vector.tensor_mul(
                            qTp[:, sl], qTp[:, sl], rsqf_f[:, sl]
                        )

                    # attention (S^T layout, no max-subtraction); softmax
                    # denominator accumulated on DVE + Pool partition-reduce
                    # instead of PE ones-matmuls
                    def attn(qs):
                        for hh in range(HPC):
                            nk = 4 * qs + 4
                            PT = ptp.tile([P, NT, 512], BF16, tag="PT")
                            # two partial accumulators: even k-tiles on DVE,
                            # odd on Pool — splits the add load across engines
                            dsum = dsp.tile([P, 512], F32, tag="dsum")
                            dsumB = dsp.tile([P, 512], F32, tag="dsumB")
                            for kt in range(nk):
                                # columns 0..r*128 are fully causal-masked:
                                # skip them in every matmul of this k-tile
                                r = kt - 4 * qs
                                c0 = max(r, 0) * 128
                                ps_s = psA.tile([P, 512], F32, tag="ps_s")
                                nc.tensor.matmul(
                                    ps_s[:, c0:512],
                                    kTn[:, hh, kt * 128 : (kt + 1) * 128],
                                    qTn[:, hh, qs * 512 + c0 : (qs + 1) * 512],
                                    start=True,
                                    stop=False,
                                )
                                hb = hh * ROPE
                                nc.tensor.matmul(
                                    ps_s[:, c0:512],
                                    kpe[
                                        hb : hb + ROPE, kt // 2,
                                        (kt % 2) * 128 : (kt % 2) * 128 + 128,
                                    ],
                                    qTp[hb : hb + ROPE, qs * 512 + c0 : (qs + 1) * 512],
                                    start=False,
                                    stop=True,
                                )
                                if c0 > 0:
                                    nc.vector.memset(PT[:, kt, 0:c0], 0.0)
                                nc.scalar.activation(
                                    PT[:, kt, c0:512], ps_s[:, c0:512], AF.Exp
                                )
                                if 0 <= r <= 3:
                                    nc.vector.tensor_mul(
                                        PT[:, kt, r * 128 : (r + 1) * 128],
                                        PT[:, kt, r * 128 : (r + 1) * 128],
                                        tri_sb[:],
                                    )
                                eng = nc.vector if kt % 2 == 0 else nc.gpsimd
                                acc = dsum if kt % 2 == 0 else dsumB
                                if kt < 2:
                                    eng.tensor_copy(acc[:], PT[:, kt, :])
                                else:
                                    eng.tensor_add(acc[:], acc[:], PT[:, kt, :])
                            ps_o = psB.tile([P, 512], F32, tag="bp")
                            for kt in range(nk):
                                c0 = max(kt - 4 * qs, 0) * 128
                                nc.tensor.matmul(
                                    ps_o[:, c0:512],
                                    vtok[:, kt, hh * VD : (hh + 1) * VD],
                                    PT[:, kt, c0:512],
                                    start=(kt == 0),
                                    stop=(kt == nk - 1),
                                )
                            dred = dsp.tile([P, 512], F32, tag="dred")
                            nc.vector.tensor_add(dsum[:], dsum[:], dsumB[:])
                            nc.gpsimd.partition_all_reduce(
                                dred[:], dsum[:], channels=128,
                                reduce_op=bass_isa.ReduceOp.add,
                            )
                            rec = rcp.tile([P, 512], F32, tag="rec")
                            nc.vector.reciprocal(rec[:], dred[:])
                            nc.vector.tensor_mul(
                                OnT[:, hh, qs * 512 : (qs + 1) * 512], ps_o[:], rec[:]
                            )

                    def wo_block(qs):
                        for tt in range(4 * qs, 4 * qs + 4):
                            for half in range(2):
                                oc = ocp.tile([P, HID // 2], BF16, tag="oc")
                                for hc in range(NHS // 2):  # 5 x 512-col chunks
                                    hs = half * (NHS // 2) + hc
                                    ps_f = psB.tile([P, 512], F32, tag="bp")
                                    nc.tensor.matmul(
                                        ps_f[:],
                                        OnT[:, 0, tt * 128 : (tt + 1) * 128],
                                        wo_sb[:, 0, hs * 512 : (hs + 1) * 512],
                                        start=True,
                                        stop=False,
                                    )
                                    nc.tensor.matmul(
                                        ps_f[:],
                                        OnT[:, 1, tt * 128 : (tt + 1) * 128],
                                        wo_sb[:, 1, hs * 512 : (hs + 1) * 512],
                                        start=False,
                                        stop=True,
                                    )
                                    # alternate copy engine so copies keep
                                    # pace with the matmul stream
                                    if hc % 2 == 0:
                                        nc.scalar.copy(
                                            oc[:, hc * 512 : (hc + 1) * 512], ps_f[:]
                                        )
                                    else:
                                        nc.vector.tensor_copy(
                                            oc[:, hc * 512 : (hc + 1) * 512], ps_f[:]
                                        )
                                nc.sync.dma_start(
                                    out[
                                        tt * 128 : (tt + 1) * 128,
                                        half * (HID // 2) : (half + 1) * (HID // 2),
                                    ],
                                    oc[:],
                                )

                    # interleave pe-rope slices, attention and w_o so DVE
                    # rope/divide latency hides behind PE matmul streams
                    q_pe_slice(0)
                    attn(0)
                    q_pe_slice(1)
                    wo_block(0)
                    attn(1)
                    q_pe_slice(2)
                    wo_block(1)
                    attn(2)
                    q_pe_slice(3)
                    wo_block(2)
                    attn(3)
                    wo_block(3)

    nc.compile()
    return nc


_NC_CACHE = None


def _get_nc():
    global _NC_CACHE
    if _NC_CACHE is None:
        _NC_CACHE = _build_nc()
    return _NC_CACHE


def _bf(x):
    return np.ascontiguousarray(x.astype(BF))


def _prep_in_maps(inputs):
    hidden = np.asarray(inputs["hidden_states"], dtype=np.float32)
    w_q_a = np.asarray(inputs["w_q_a"], dtype=np.float32)
    q_a_norm_w = np.asarray(inputs["q_a_norm_w"], dtype=np.float32)
    w_q_b = np.asarray(inputs["w_q_b"], dtype=np.float32)
    w_kv_a = np.asarray(inputs["w_kv_a"], dtype=np.float32)
    kv_a_norm_w = np.asarray(inputs["kv_a_norm_w"], dtype=np.float32)
    w_kv_b = np.asarray(inputs["w_kv_b"], dtype=np.float32)
    w_o = np.asarray(inputs["w_o"], dtype=np.float32)
    pos = np.asarray(inputs["positions"]).astype(np.float32)

    # rope tables, feature-major, evens/odds share the same row index
    inv_freq = _yarn_inv_freq()
    freqs = pos[:, None] * inv_freq[None, :]          # [T, 32]
    cosf = np.cos(freqs).T * COS_SIN_MSCALE           # [32, T]
    sinf = np.sin(freqs).T * COS_SIN_MSCALE
    cosf_b, sinf_b = _bf(cosf), _bf(sinf)
    cosl2 = np.concatenate([cosf_b, cosf_b], 0)       # duplicated halves
    sinl2 = np.concatenate([sinf_b, sinf_b], 0)

    # a-proj weights: [17 mtiles, 128p, 40k, 128c], pe cols de-interleaved
    wkva_pe = w_kv_a[:, KL:][:, PE_PERM]
    wa_full = np.concatenate(
        [w_q_a, w_kv_a[:, :KL], wkva_pe, np.zeros((HID, 64), np.float32)], axis=1
    )  # [5120, 2176]
    wa_l = _bf(wa_full.reshape(HCH, P, MT, P).transpose(2, 1, 0, 3))

    # fold RMSNorm gains + attention scale into b-proj weights
    wqb_s = w_q_b * q_a_norm_w[:, None] * ATTN_SCALE
    wkvb_s = w_kv_b * kv_a_norm_w[:, None]

    ones_b = _bf(np.ones((P, P), np.float32))
    tri_b = _bf(np.triu(np.ones((P, P), np.float32)))

    shared = {
        "wa": wa_l,
        "cosf": cosf_b,
        "sinf": sinf_b,
        "ones": ones_b,
        "tri": tri_b,
    }

    in_maps = []
    for c in range(NCORE):
        h0 = HPC * c
        # hidden slice, feature-major [128, 40, 256]
        hs = hidden[c * TLOC : (c + 1) * TLOC, :]
        hT_l = _bf(hs.T.reshape(HCH, P, TLOC).transpose(1, 0, 2))
        # w_q_b cols for this core's heads: [h0 nope | h1 nope | h0 pe | h1 pe]
        nope_cols, pe_cols = [], []
        for h in range(h0, h0 + HPC):
            blk = wqb_s[:, h * QK : (h + 1) * QK]
            nope_cols.append(blk[:, :NOPE])
            pe_cols.append(blk[:, NOPE:][:, PE_PERM])
        wqb_core = np.concatenate(nope_cols + pe_cols, axis=1)  # [1536, 384]
        wqb_l = _bf(wqb_core.reshape(QLC, P, HPC * QK).transpose(1, 0, 2))
        # w_kv_b cols: [h0 nope, h1 nope, h0 v, h1 v]
        nopes = [
            wkvb_s[:, h * (NOPE + VD) : h * (NOPE + VD) + NOPE]
            for h in range(h0, h0 + HPC)
        ]
        vs = [
            wkvb_s[:, h * (NOPE + VD) + NOPE : (h + 1) * (NOPE + VD)]
            for h in range(h0, h0 + HPC)
        ]
        wkvb_core = np.concatenate(nopes + vs, axis=1)  # [512, 512]
        wkvb_l = _bf(wkvb_core.reshape(KLC, P, 512).transpose(1, 0, 2))
        # w_o rows for this core's heads: [128, 2, 5120]
        wo_core = w_o[h0 * VD : (h0 + HPC) * VD, :]
        wo_l = _bf(wo_core.reshape(HPC, P, HID).transpose(1, 0, 2))

        m = dict(shared)
        m.update(
            {
                "hT": hT_l,
                "wqb": wqb_l,
                "wkvb": wkvb_l,
                "wo": wo_l,
                "cosl": np.ascontiguousarray(cosl2[:, c * TLOC : (c + 1) * TLOC]),
                "sinl": np.ascontiguousarray(sinl2[:, c * TLOC : (c + 1) * TLOC]),
            }
        )
        in_maps.append(m)
    return in_maps


def kernel(**inputs):
    global LAST_EXEC_NS
    nc = _get_nc()
    in_maps = _prep_in_maps(inputs)
    trace = os.environ.get("KERNEL_TRACE", "0") == "1"
    res = run_bass_kernel_spmd(
        nc, in_maps, core_ids=list(range(NCORE)), trace=trace
    )
    LAST_EXEC_NS = res.exec_time_ns
    out = res.results[0]["out"].astype(np.float32)
    for i in range(1, NCORE):
        out += res.results[i]["out"].astype(np.float32)
    return out



# revision 38
# speedup vs baseline: 1.1039x; 1.0340x over previous
"""DeepSeek MLA prefill on 8 TRN2 NeuronCores.

Sharding: tensor-parallel over heads (2 heads/core) for the b-projections,
attention and w_o (row-parallel -> host sums partials); sequence-parallel
a-projections (each core computes 256 tokens of q_a/kv_a/k_pe, normalizes,
ropes k_pe, then on-device AllGathers replicate the 2112x256 activations).
The kv-group a-proj runs first so its (small) gather and the whole kv
b-projection overlap the q-group a-proj and gather.

All activations that feed matmuls are kept feature-major ([d, T]) so no
on-device transposes are needed; v is produced token-major directly.
Matmuls run in bf16 with f32 PSUM accumulation (rel-err gate is ~2e-2).
"""

import math
import os

import ml_dtypes
import numpy as np

import concourse.bacc as bacc
import concourse.bass_isa as bass_isa
import concourse.mybir as mybir
import concourse.tile as tile
from concourse.bass_utils import run_bass_kernel_spmd

F32 = mybir.dt.float32
BF16 = mybir.dt.bfloat16
AF = mybir.ActivationFunctionType
ALU = mybir.AluOpType

# problem dims (hardcoded per contract)
T, HID, H = 2048, 5120, 16
QL, KL = 1536, 512
NOPE, ROPE, VD = 128, 64, 128
QK = NOPE + ROPE
EPS = 1e-6
NCORE = 8
HPC = H // NCORE          # heads per core = 2
TLOC = T // NCORE         # tokens per core = 256
P = 128
HCH = HID // P            # 40 hidden chunks
QLC = QL // P             # 12
KLC = KL // P             # 4
MT = QLC + KLC + 1        # 17 a-proj output tiles (12 q + 4 kv + 1 pe[64])
NKV = MT - QLC            # 5 kv-group tiles
NT = T // P               # 16 token tiles
NQS = 4                   # 512-wide q slices per head
NHS = HID // 512          # 10 output column slices

# yarn rope params
BASE, FACTOR = 10000.0, 40.0
BETA_FAST, BETA_SLOW, ORIG_MAX = 32.0, 1.0, 4096
MSCALE = 1.0
MSCALE_ALL_DIM = 1.0


def _yarn_get_mscale(scale, m):
    if scale <= 1.0:
        return 1.0
    return 0.1 * m * math.log(scale) + 1.0


def _yarn_inv_freq():
    pos_freqs = BASE ** (np.arange(0, ROPE, 2, dtype=np.float64) / ROPE)
    extra = 1.0 / pos_freqs
    inter = 1.0 / (FACTOR * pos_freqs)

    def corr_dim(n):
        return ROPE * math.log(ORIG_MAX / (n * 2 * math.pi)) / (2 * math.log(BASE))

    low = max(math.floor(corr_dim(BETA_FAST)), 0)
    high = min(math.ceil(corr_dim(BETA_SLOW)), ROPE - 1)
    ramp = np.clip(
        (np.arange(ROPE // 2, dtype=np.float64) - low) / max(high - low, 0.001),
        0.0,
        1.0,
    )
    mask = 1.0 - ramp
    return (inter * (1.0 - mask) + extra * mask).astype(np.float32)


COS_SIN_MSCALE = _yarn_get_mscale(FACTOR, MSCALE) / _yarn_get_mscale(
    FACTOR, MSCALE_ALL_DIM
)
_M = _yarn_get_mscale(FACTOR, MSCALE_ALL_DIM)
ATTN_SCALE = (QK ** -0.5) * _M * _M

BF = ml_dtypes.bfloat16
# de-interleave perm: even rope dims then odd rope dims
PE_PERM = np.concatenate([np.arange(0, ROPE, 2), np.arange(1, ROPE, 2)])

LAST_EXEC_NS = None


def _build_nc(single=False, reps=1):
    # single=True: no collective, 1 core — for cost-model timeline sims only
    nc = bacc.Bacc(
        "TRN2",
        target_bir_lowering=False,
        debug=False,
        num_devices=1 if single else NCORE,
    )

    hT = nc.dram_tensor("hT", [P, HCH, TLOC], BF16, kind="ExternalInput").ap()
    wa = nc.dram_tensor("wa", [MT, P, HCH, P], BF16, kind="ExternalInput").ap()
    wqb = nc.dram_tensor("wqb", [P, QLC, HPC * QK], BF16, kind="ExternalInput").ap()
    wkvb = nc.dram_tensor("wkvb", [P, KLC, 512], BF16, kind="ExternalInput").ap()
    wo = nc.dram_tensor("wo", [P, HPC, HID], BF16, kind="ExternalInput").ap()
    cosf = nc.dram_tensor("cosf", [ROPE // 2, T], BF16, kind="ExternalInput").ap()
    sinf = nc.dram_tensor("sinf", [ROPE // 2, T], BF16, kind="ExternalInput").ap()
    cosl = nc.dram_tensor("cosl", [ROPE, TLOC], BF16, kind="ExternalInput").ap()
    sinl = nc.dram_tensor("sinl", [ROPE, TLOC], BF16, kind="ExternalInput").ap()
    onesd = nc.dram_tensor("ones", [P, P], BF16, kind="ExternalInput").ap()
    trid = nc.dram_tensor("tri", [P, P], BF16, kind="ExternalInput").ap()
    out = nc.dram_tensor("out", [T, HID], BF16, kind="ExternalOutput").ap()

    locb = nc.dram_tensor("locb", [MT, P, TLOC], BF16).ap()
    locr = nc.dram_tensor("locr", [P, TLOC], F32).ap()
    gathkv = nc.dram_tensor(
        "gathkv", [NCORE, NKV, P, TLOC], BF16, addr_space="Shared"
    ).ap()
    # q latents gathered RAW (pre-norm) in 2-mtile chunks as the a-proj
    # produces them; the rsq row-norms gather separately and are applied
    # after the q b-projection (scaling commutes through the matmul)
    gathq = [
        nc.dram_tensor(f"gathq{i}", [NCORE, 2, P, TLOC], BF16, addr_space="Shared").ap()
        for i in range(QLC // 2)
    ]
    gathr = nc.dram_tensor("gathr", [NCORE, P, TLOC], F32, addr_space="Shared").ap()

    with tile.TileContext(nc) as tc:
        with (
            tc.tile_pool(name="const", bufs=1) as cp,
            tc.tile_pool(name="persist", bufs=1) as pp,
            tc.tile_pool(name="ocp", bufs=3) as ocp,
        ):
            ones_sb = cp.tile([P, P], BF16, tag="ones")
            tri_sb = cp.tile([P, P], BF16, tag="tri")
            cosf_sb = cp.tile([ROPE // 2, T], BF16, tag="cosf")
            sinf_sb = cp.tile([ROPE // 2, T], BF16, tag="sinf")
            cosl_sb = cp.tile([ROPE, TLOC], BF16, tag="cosl")
            sinl_sb = cp.tile([ROPE, TLOC], BF16, tag="sinl")
            eps_sb = cp.tile([P, 1], F32, tag="eps")
            nc.vector.memset(eps_sb[:], EPS)

            # persistent attention operands (live across the phase transition)
            qTn = pp.tile([P, HPC, T], BF16, tag="qTn")
            # both heads' roped q_pe packed: rows [h0e h0o h1e h1o] x 32
            qTp = pp.tile([P, T], BF16, tag="qTp")
            kTn = pp.tile([P, HPC, T], BF16, tag="kTn")
            vtok = pp.tile([P, NT, HPC * VD], BF16, tag="vtok")
            OnT = pp.tile([P, HPC, T], BF16, tag="OnT")
            # k_pe duplicated into both 64-row halves so each head's score
            # matmul has lhsT/rhs at the same base partition (0 or 64)
            kpe = pp.tile([P, NCORE, TLOC], BF16, tag="kpe")
            wkvb_sb = pp.tile([P, KLC, 512], BF16, tag="wkvb")
            wqb_sb = pp.tile([P, QLC, HPC * QK], BF16, tag="wqb")
            # raw q latents (all cores) + per-token rsq, filled during phase 1
            qag = pp.tile([P, QLC, NCORE, TLOC], BF16, tag="qag")
            qag_f = qag.rearrange("p m c t -> p m (c t)")
            rsqf = pp.tile([P, NCORE, TLOC], F32, tag="rsqf")
            rsqf_f = rsqf.rearrange("p c t -> p (c t)")

            for _rep in range(reps):
                # ---------------- phase 1: local a-projections ----------------
                with (
                    tc.tile_pool(name="p1", bufs=1) as p1,
                    tc.tile_pool(name="wap", bufs=2) as wap,
                    tc.tile_pool(name="sqp", bufs=3) as sqp,
                    tc.tile_pool(name="ps1", bufs=3, space="PSUM") as ps1,
                    tc.tile_pool(name="pss", bufs=1, space="PSUM") as pss,
                ):
                    hT_sb = p1.tile([P, HCH, TLOC], BF16, tag="hT")
                    # only the first tiny chunk up front; the rest is
                    # interleaved with the first weight tile's chunks below
                    nc.sync.dma_start(hT_sb[:, 0:2, :], hT[:, 0:2, :])
                    araw = p1.tile([P, MT, TLOC], BF16, tag="araw")
                    anrm = p1.tile([P, NKV, TLOC], BF16, tag="anrm")
                    # kv gather buffer lives only in phase 1 (kv_b consumes it)
                    kag = p1.tile([P, KLC, NCORE, TLOC], BF16, tag="kag")
                    kag_f = kag.rearrange("p m c t -> p m (c t)")
                    ssq = pss.tile([P, TLOC], F32, tag="ssq")
                    sskv = pss.tile([P, TLOC], F32, tag="sskv")

                    pend_sq = None

                    def emit_ssq(sq, sm):
                        if sm < QLC:
                            nc.tensor.matmul(
                                ssq[:], ones_sb[:], sq[:],
                                start=(sm == 0), stop=(sm == QLC - 1),
                                skip_group_check=True,
                            )
                        else:
                            nc.tensor.matmul(
                                sskv[:], ones_sb[:], sq[:],
                                start=(sm == QLC), stop=(sm == QLC + KLC - 1),
                                skip_group_check=True,
                            )

                    # kv-group mtiles first so their collective + the whole kv
                    # b-projection overlap the (3x bigger) q-group a-proj
                    for m in list(range(QLC, MT)) + list(range(QLC)):
                        wt = wap.tile([P, HCH, P], BF16, tag="wt")
                        if m == QLC:  # first mtile: pace-matched interleave of
                            # weight chunks and the rest of hT so the k-loop
                            # never waits long on either stream
                            nc.sync.dma_start(wt[:, 0:5, :], wa[m, :, 0:5, :])
                            nc.sync.dma_start(ones_sb[:], onesd)
                            nc.sync.dma_start(cosl_sb[:], cosl)
                            nc.sync.dma_start(sinl_sb[:], sinl)
                            for (w0, w1), (h0, h1) in [
                                ((5, 14), (2, 8)),
                                ((14, 24), (8, 16)),
                                ((24, 40), (16, 28)),
                                ((40, 40), (28, 40)),
                            ]:
                                if w1 > w0:
                                    nc.sync.dma_start(
                                        wt[:, w0:w1, :], wa[m, :, w0:w1, :]
                                    )
                                nc.sync.dma_start(hT_sb[:, h0:h1, :], hT[:, h0:h1, :])
                        else:
                            nc.sync.dma_start(wt[:], wa[m])
                            if m == QLC - 2:
                                # phase-2 q weights: late enough to not crowd
                                # the wa stream, early enough for phase 2
                                nc.sync.dma_start(wqb_sb[:], wqb)
                        ps = ps1.tile([P, TLOC], F32, tag="aps")
                        for k in range(HCH):
                            nc.tensor.matmul(
                                ps[:],
                                wt[:, k, :],
                                hT_sb[:, k, :],
                                start=(k == 0),
                                stop=(k == HCH - 1),
                            )
                        nc.scalar.copy(araw[:, m, :], ps[:])
                        # sum-of-squares matmuls run one mtile late so the
                        # Act copy+Square chain hides under the next a-proj
                        if pend_sq is not None:
                            emit_ssq(*pend_sq)
                            pend_sq = None
                        if m < QLC + KLC:
                            sq = sqp.tile([P, TLOC], BF16, tag="sq")
                            nc.scalar.activation(sq[:], ps[:], AF.Square)
                            pend_sq = (sq, m)

                        if m < QLC and m % 2 == 1:
                            if m == QLC - 1:
                                # final ssq + rsq chain first: the tiny rsq
                                # gather must beat the last chunk's gathers
                                # into the DMA queues (phase 2 needs it first)
                                emit_ssq(*pend_sq)
                                pend_sq = None
                                rsq_q = p1.tile([P, TLOC], F32, tag="rsq_q")
                                tmpf = p1.tile([P, TLOC], F32, tag="tmpf")
                                nc.scalar.activation(
                                    tmpf[:], ssq[:], AF.Sqrt,
                                    bias=eps_sb[:], scale=1.0 / QL,
                                )
                                nc.vector.reciprocal(rsq_q[:], tmpf[:])
                                nc.sync.dma_start(locr, rsq_q[:])
                                if not single:
                                    nc.gpsimd.collective_compute(
                                        "AllGather",
                                        ALU.bypass,
                                        replica_groups=[list(range(NCORE))],
                                        ins=[locr.opt()],
                                        outs=[gathr.opt()],
                                    )
                                nc.sync.dma_start(
                                    rsqf[:], gathr.rearrange("c p t -> p c t")
                                )
                            # ship this RAW q-latent pair: write, gather, and
                            # pull into SBUF while later mtiles still compute
                            cch = m // 2
                            nc.sync.dma_start(
                                locb[m - 1 : m + 1].rearrange("m p t -> p m t"),
                                araw[:, m - 1 : m + 1, :],
                            )
                            if not single:
                                nc.gpsimd.collective_compute(
                                    "AllGather",
                                    ALU.bypass,
                                    replica_groups=[list(range(NCORE))],
                                    ins=[locb[m - 1 : m + 1].opt()],
                                    outs=[gathq[cch].opt()],
                                )
                            for j in range(2):
                                nc.sync.dma_start(
                                    qag[:, m - 1 + j],
                                    gathq[cch][:, j].rearrange("c p t -> p c t"),
                                )

                        if m == MT - 1:
                            # kv group locally complete: normalize, rope, ship
                            rsq_k = p1.tile([P, TLOC], F32, tag="rsq_k")
                            tmpf2 = p1.tile([P, TLOC], F32, tag="tmpf2")
                            nc.scalar.activation(
                                tmpf2[:], sskv[:], AF.Sqrt,
                                bias=eps_sb[:], scale=1.0 / KL,
                            )
                            nc.vector.reciprocal(rsq_k[:], tmpf2[:])
                            for mm in range(QLC, QLC + KLC):
                                nc.vector.tensor_mul(
                                    anrm[:, mm - QLC, :], araw[:, mm, :], rsq_k[:]
                                )
                            # rope k_pe (rows 0:32 even, 32:64 odd of tile MT-1).
                            # Two-SBUF-input ops must share base partition, so
                            # cos/sin tables are duplicated across both halves.
                            t1 = p1.tile([ROPE, TLOC], BF16, tag="t1")
                            t2 = p1.tile([ROPE, TLOC], BF16, tag="t2")
                            xe = araw[0:32, MT - 1, :]
                            xo = araw[32:64, MT - 1, :]
                            nc.vector.tensor_mul(t1[0:32, :], xe, cosl_sb[0:32, :])
                            nc.vector.tensor_mul(t2[0:32, :], xo, sinl_sb[32:64, :])
                            nc.vector.tensor_sub(
                                anrm[0:32, NKV - 1, :], t1[0:32, :], t2[0:32, :]
                            )
                            nc.vector.tensor_mul(t1[32:64, :], xo, cosl_sb[32:64, :])
                            nc.vector.tensor_mul(t2[32:64, :], xe, sinl_sb[0:32, :])
                            nc.vector.tensor_add(
                                anrm[32:64, NKV - 1, :], t1[32:64, :], t2[32:64, :]
                            )
                            nc.vector.memset(anrm[64:128, NKV - 1, :], 0.0)
                            nc.sync.dma_start(
                                locb[QLC:MT].rearrange("m p t -> p m t"),
                                anrm[:, 0:NKV, :],
                            )
                            if not single:
                                nc.gpsimd.collective_compute(
                                    "AllGather",
                                    ALU.bypass,
                                    replica_groups=[list(range(NCORE))],
                                    ins=[locb[QLC:MT].opt()],
                                    outs=[gathkv.opt()],
                                )
                            # kv gather-in + whole kv b-projection — overlaps
                            # the q-group a-proj matmuls still streaming on PE
                            nc.sync.dma_start(wkvb_sb[:], wkvb)
                            for mm in range(KLC):
                                nc.sync.dma_start(
                                    kag[:, mm],
                                    gathkv[:, mm].rearrange("c p t -> p c t"),
                                )
                            for half in range(2):
                                nc.sync.dma_start(
                                    kpe[half * ROPE : (half + 1) * ROPE],
                                    gathkv[:, NKV - 1, 0:ROPE, :].rearrange(
                                        "c p t -> p c t"
                                    ),
                                )
                            # k_nope^T per head: [128, T]
                            for hh in range(HPC):
                                for s in range(4):
                                    psb = ps1.tile([P, 512], F32, tag="bp")
                                    for k in range(KLC):
                                        nc.tensor.matmul(
                                            psb[:],
                                            wkvb_sb[:, k, hh * 128 : (hh + 1) * 128],
                                            kag_f[:, k, s * 512 : (s + 1) * 512],
                                            start=(k == 0),
                                            stop=(k == KLC - 1),
                                        )
                                    # Pool engine: keeps Act free for the
                                    # copy+Square chain feeding the ssq matmuls
                                    nc.gpsimd.tensor_copy(
                                        kTn[:, hh, s * 512 : (s + 1) * 512], psb[:]
                                    )
                            # v token-major: [t, 2*VD] per token tile
                            for tt in range(NT):
                                psb = ps1.tile([P, 512], F32, tag="bp")
                                for k in range(KLC):
                                    nc.tensor.matmul(
                                        psb[:, 0 : HPC * VD],
                                        kag[
                                            :, k, tt // 2,
                                            (tt % 2) * 128 : (tt % 2) * 128 + 128,
                                        ],
                                        wkvb_sb[:, k, 256:512],
                                        start=(k == 0),
                                        stop=(k == KLC - 1),
                                    )
                                nc.vector.tensor_copy(
                                    vtok[:, tt, :], psb[:, 0 : HPC * VD]
                                )

                    if pend_sq is not None:
                        emit_ssq(*pend_sq)
                        pend_sq = None

                # ---------------- phase 2: q b-proj + attention + w_o ----------
                with (
                    tc.tile_pool(name="p2", bufs=1) as p2,
                    tc.tile_pool(name="ptp", bufs=2) as ptp,
                    tc.tile_pool(name="rcp", bufs=2) as rcp,
                    tc.tile_pool(name="dsp", bufs=2) as dsp,
                    tc.tile_pool(name="psB", bufs=3, space="PSUM") as psB,
                    tc.tile_pool(name="psA", bufs=3, space="PSUM") as psA,
                ):
                    # w_o weights only live in phase 2
                    wo_sb = p2.tile([P, HPC, HID], BF16, tag="wo")
                    nc.sync.dma_start(cosf_sb[:], cosf)
                    nc.sync.dma_start(sinf_sb[:], sinf)
                    nc.sync.dma_start(tri_sb[:], trid)
                    nc.sync.dma_start(wo_sb[:], wo)

                    # q^T: nope [128, T] per head; both heads' pe packed M=128
                    # (wqb cols: [h0 nope | h1 nope | h0 pe | h1 pe]);
                    # rsq row-norm folded into the PSUM->SBUF move
                    def nope_slice(s):
                        for hh in range(HPC):
                            ps = psB.tile([P, 512], F32, tag="bp")
                            for k in range(QLC):
                                nc.tensor.matmul(
                                    ps[:],
                                    wqb_sb[:, k, hh * NOPE : (hh + 1) * NOPE],
                                    qag_f[:, k, s * 512 : (s + 1) * 512],
                                    start=(k == 0),
                                    stop=(k == QLC - 1),
                                )
                            nc.vector.tensor_mul(
                                qTn[:, hh, s * 512 : (s + 1) * 512],
                                ps[:],
                                rsqf_f[:, s * 512 : (s + 1) * 512],
                            )

                    def q_pe_slice(s):
                        ps = psB.tile([P, 512], F32, tag="bp")
                        for k in range(QLC):
                            nc.tensor.matmul(
                                ps[:],
                                wqb_sb[:, k, HPC * NOPE : HPC * QK],
                                qag_f[:, k, s * 512 : (s + 1) * 512],
                                start=(k == 0),
                                stop=(k == QLC - 1),
                            )
                        # rope both heads' pe straight out of PSUM
                        # (PSUM x SBUF ops are exempt from the equal-base rule)
                        sl = slice(s * 512, (s + 1) * 512)
                        cs, sn = cosf_sb[:, sl], sinf_sb[:, sl]
                        rt = p2.tile([P, 512], BF16, tag="rt")
                        for hh in range(HPC):
                            b = hh * ROPE
                            xe, xo = ps[b : b + 32, :], ps[b + 32 : b + 64, :]
                            nc.vector.tensor_mul(qTp[b : b + 32, sl], xe, cs)
                            nc.vector.tensor_mul(rt[b : b + 32, :], xo, sn)
                            nc.vector.tensor_sub(
                                qTp[b : b + 32, sl],
                                qTp[b : b + 32, sl],
                                rt[b : b + 32, :],
                            )
                            nc.vector.tensor_mul(qTp[b + 32 : b + 64, sl], xo, cs)
                            nc.vector.tensor_mul(rt[b + 32 : b + 64, :], xe, sn)
                            nc.vector.tensor_add(
                                qTp[b + 32 : b + 64, sl],
                                qTp[b + 32 : b + 64, sl],
                                rt[b + 32 : b + 64, :],
                            )
                        # apply the q rsq row-norm (commutes with rope)
                        nc.vector.tensor_mul(
                            qTp[:, sl], qTp[:, sl], rsqf_f[:, sl]
                        )

                    # attention (S^T layout, no max-subtraction); softmax
                    # denominator accumulated on DVE + Pool partition-reduce
                    # instead of PE ones-matmuls
                    def attn(qs):
                        for hh in range(HPC):
                            nk = 4 * qs + 4
                            PT = ptp.tile([P, NT, 512], BF16, tag="PT")
                            # two partial accumulators: even k-tiles on DVE,
                            # odd on Pool — splits the add load across engines
                            dsum = dsp.tile([P, 512], F32, tag="dsum")
                            dsumB = dsp.tile([P, 512], F32, tag="dsumB")
                            for kt in range(nk):
                                # columns 0..r*128 are fully causal-masked:
                                # skip them in every matmul of this k-tile
                                r = kt - 4 * qs
                                c0 = max(r, 0) * 128
                                ps_s = psA.tile([P, 512], F32, tag="ps_s")
                                nc.tensor.matmul(
                                    ps_s[:, c0:512],
                                    kTn[:, hh, kt * 128 : (kt + 1) * 128],
                                    qTn[:, hh, qs * 512 + c0 : (qs + 1) * 512],
                                    start=True,
                                    stop=False,
                                )
                                hb = hh * ROPE
                                nc.tensor.matmul(
                                    ps_s[:, c0:512],
                                    kpe[
                                        hb : hb + ROPE, kt // 2,
                                        (kt % 2) * 128 : (kt % 2) * 128 + 128,
                                    ],
                                    qTp[hb : hb + ROPE, qs * 512 + c0 : (qs + 1) * 512],
                                    start=False,
                                    stop=True,
                                )
                                if c0 > 0:
                                    nc.vector.memset(PT[:, kt, 0:c0], 0.0)
                                nc.scalar.activation(
                                    PT[:, kt, c0:512], ps_s[:, c0:512], AF.Exp
                                )
                                if 0 <= r <= 3:
                                    nc.vector.tensor_mul(
                                        PT[:, kt, r * 128 : (r + 1) * 128],
                                        PT[:, kt, r * 128 : (r + 1) * 128],
                                        tri_sb[:],
                                    )
                                eng = nc.vector if kt % 2 == 0 else nc.gpsimd
                                acc = dsum if kt % 2 == 0 else dsumB
                                if kt < 2:
                                    eng.tensor_copy(acc[:], PT[:, kt, :])
                                else:
                                    eng.tensor_add(acc[:], acc[:], PT[:, kt, :])
                            ps_o = psB.tile([P, 512], F32, tag="bp")
                            for kt in range(nk):
                                c0 = max(kt - 4 * qs, 0) * 128
                                nc.tensor.matmul(
                                    ps_o[:, c0:512],
                                    vtok[:, kt, hh * VD : (hh + 1) * VD],
                                    PT[:, kt, c0:512],
                                    start=(kt == 0),
                                    stop=(kt == nk - 1),
                                )
                            dred = dsp.tile([P, 512], F32, tag="dred")
                            nc.vector.tensor_add(dsum[:], dsum[:], dsumB[:])
                            nc.gpsimd.partition_all_reduce(
                                dred[:], dsum[:], channels=128,
                                reduce_op=bass_isa.ReduceOp.add,
                            )
                            rec = rcp.tile([P, 512], F32, tag="rec")
                            nc.vector.reciprocal(rec[:], dred[:])
                            nc.vector.tensor_mul(
                                OnT[:, hh, qs * 512 : (qs + 1) * 512], ps_o[:], rec[:]
                            )

                    def wo_block(qs):
                        for tt in range(4 * qs, 4 * qs + 4):
                            for half in range(2):
                                oc = ocp.tile([P, HID // 2], BF16, tag="oc")
                                for hc in range(NHS // 2):  # 5 x 512-col chunks
                                    hs = half * (NHS // 2) + hc
                                    ps_f = psB.tile([P, 512], F32, tag="bp")
                                    nc.tensor.matmul(
                                        ps_f[:],
                                        OnT[:, 0, tt * 128 : (tt + 1) * 128],
                                        wo_sb[:, 0, hs * 512 : (hs + 1) * 512],
                                        start=True,
                                        stop=False,
                                    )
                                    nc.tensor.matmul(
                                        ps_f[:],
                                        OnT[:, 1, tt * 128 : (tt + 1) * 128],
                                        wo_sb[:, 1, hs * 512 : (hs + 1) * 512],
                                        start=False,
                                        stop=True,
                                    )
                                    # alternate copy engine so copies keep
                                    # pace with the matmul stream
                                    if hc % 2 == 0:
                                        nc.scalar.copy(
                                            oc[:, hc * 512 : (hc + 1) * 512], ps_f[:]
                                        )
                                    else:
                                        nc.vector.tensor_copy(
                                            oc[:, hc * 512 : (hc + 1) * 512], ps_f[:]
                                        )
                                nc.sync.dma_start(
                                    out[
                                        tt * 128 : (tt + 1) * 128,
                                        half * (HID // 2) : (half + 1) * (HID // 2),
                                    ],
                                    oc[:],
                                )

                    # staggered schedule: attention trails the q b-proj by one
                    # slice and w_o trails attention by one q-slice, so the
                    # serial DVE chains (qTn muls, rope, softmax divide) always
                    # have a full PE block of cover
                    nope_slice(0)
                    q_pe_slice(0)
                    nope_slice(1)
                    q_pe_slice(1)
                    attn(0)
                    nope_slice(2)
                    q_pe_slice(2)
                    attn(1)
                    wo_block(0)
                    nope_slice(3)
                    q_pe_slice(3)
                    attn(2)
                    wo_block(1)
                    attn(3)
                    wo_block(2)
                    wo_block(3)

    nc.compile()
    return nc


_NC_CACHE = None


def _get_nc():
    global _NC_CACHE
    if _NC_CACHE is None:
        _NC_CACHE = _build_nc()
    return _NC_CACHE


def _bf(x):
    return np.ascontiguousarray(x.astype(BF))


def _prep_in_maps(inputs):
    hidden = np.asarray(inputs["hidden_states"], dtype=np.float32)
    w_q_a = np.asarray(inputs["w_q_a"], dtype=np.float32)
    q_a_norm_w = np.asarray(inputs["q_a_norm_w"], dtype=np.float32)
    w_q_b = np.asarray(inputs["w_q_b"], dtype=np.float32)
    w_kv_a = np.asarray(inputs["w_kv_a"], dtype=np.float32)
    kv_a_norm_w = np.asarray(inputs["kv_a_norm_w"], dtype=np.float32)
    w_kv_b = np.asarray(inputs["w_kv_b"], dtype=np.float32)
    w_o = np.asarray(inputs["w_o"], dtype=np.float32)
    pos = np.asarray(inputs["positions"]).astype(np.float32)

    # rope tables, feature-major, evens/odds share the same row index
    inv_freq = _yarn_inv_freq()
    freqs = pos[:, None] * inv_freq[None, :]          # [T, 32]
    cosf = np.cos(freqs).T * COS_SIN_MSCALE           # [32, T]
    sinf = np.sin(freqs).T * COS_SIN_MSCALE
    cosf_b, sinf_b = _bf(cosf), _bf(sinf)
    cosl2 = np.concatenate([cosf_b, cosf_b], 0)       # duplicated halves
    sinl2 = np.concatenate([sinf_b, sinf_b], 0)

    # a-proj weights: [17 mtiles, 128p, 40k, 128c], pe cols de-interleaved
    wkva_pe = w_kv_a[:, KL:][:, PE_PERM]
    wa_full = np.concatenate(
        [w_q_a, w_kv_a[:, :KL], wkva_pe, np.zeros((HID, 64), np.float32)], axis=1
    )  # [5120, 2176]
    wa_l = _bf(wa_full.reshape(HCH, P, MT, P).transpose(2, 1, 0, 3))

    # fold RMSNorm gains + attention scale into b-proj weights
    wqb_s = w_q_b * q_a_norm_w[:, None] * ATTN_SCALE
    wkvb_s = w_kv_b * kv_a_norm_w[:, None]

    ones_b = _bf(np.ones((P, P), np.float32))
    tri_b = _bf(np.triu(np.ones((P, P), np.float32)))

    shared = {
        "wa": wa_l,
        "cosf": cosf_b,
        "sinf": sinf_b,
        "ones": ones_b,
        "tri": tri_b,
    }

    in_maps = []
    for c in range(NCORE):
        h0 = HPC * c
        # hidden slice, feature-major [128, 40, 256]
        hs = hidden[c * TLOC : (c + 1) * TLOC, :]
        hT_l = _bf(hs.T.reshape(HCH, P, TLOC).transpose(1, 0, 2))
        # w_q_b cols for this core's heads: [h0 nope | h1 nope | h0 pe | h1 pe]
        nope_cols, pe_cols = [], []
        for h in range(h0, h0 + HPC):
            blk = wqb_s[:, h * QK : (h + 1) * QK]
            nope_cols.append(blk[:, :NOPE])
            pe_cols.append(blk[:, NOPE:][:, PE_PERM])
        wqb_core = np.concatenate(nope_cols + pe_cols, axis=1)  # [1536, 384]
        wqb_l = _bf(wqb_core.reshape(QLC, P, HPC * QK).transpose(1, 0, 2))
        # w_kv_b cols: [h0 nope, h1 nope, h0 v, h1 v]
        nopes = [
            wkvb_s[:, h * (NOPE + VD) : h * (NOPE + VD) + NOPE]
            for h in range(h0, h0 + HPC)
        ]
        vs = [
            wkvb_s[:, h * (NOPE + VD) + NOPE : (h + 1) * (NOPE + VD)]
            for h in range(h0, h0 + HPC)
        ]
        wkvb_core = np.concatenate(nopes + vs, axis=1)  # [512, 512]
        wkvb_l = _bf(wkvb_core.reshape(KLC, P, 512).transpose(1, 0, 2))
        # w_o rows for this core's heads: [128, 2, 5120]
        wo_core = w_o[h0 * VD : (h0 + HPC) * VD, :]
        wo_l = _bf(wo_core.reshape(HPC, P, HID).transpose(1, 0, 2))

        m = dict(shared)
        m.update(
            {
                "hT": hT_l,
                "wqb": wqb_l,
                "wkvb": wkvb_l,
                "wo": wo_l,
                "cosl": np.ascontiguousarray(cosl2[:, c * TLOC : (c + 1) * TLOC]),
                "sinl": np.ascontiguousarray(sinl2[:, c * TLOC : (c + 1) * TLOC]),
            }
        )
        in_maps.append(m)
    return in_maps


def kernel(**inputs):
    global LAST_EXEC_NS
    nc = _get_nc()
    in_maps = _prep_in_maps(inputs)
    trace = os.environ.get("KERNEL_TRACE", "0") == "1"
    res = run_bass_kernel_spmd(
        nc, in_maps, core_ids=list(range(NCORE)), trace=trace
    )
    LAST_EXEC_NS = res.exec_time_ns
    out = res.results[0]["out"].astype(np.float32)
    for i in range(1, NCORE):
        out += res.results[i]["out"].astype(np.float32)
    return out



# revision 43
# speedup vs baseline: 1.1542x; 1.0456x over previous
"""DeepSeek MLA prefill on 8 TRN2 NeuronCores.

Sharding: tensor-parallel over heads (2 heads/core) for the b-projections,
attention and w_o (row-parallel -> host sums partials); sequence-parallel
a-projections (each core computes 256 tokens of q_a/kv_a/k_pe, normalizes,
ropes k_pe, then on-device AllGathers replicate the 2112x256 activations).
The kv-group a-proj runs first so its (small) gather and the whole kv
b-projection overlap the q-group a-proj and gather.

All activations that feed matmuls are kept feature-major ([d, T]) so no
on-device transposes are needed; v is produced token-major directly.
Matmuls run in bf16 with f32 PSUM accumulation (rel-err gate is ~2e-2).
"""

import math
import os

import ml_dtypes
import numpy as np

import concourse.bacc as bacc
import concourse.bass_isa as bass_isa
import concourse.mybir as mybir
import concourse.tile as tile
from concourse.bass_utils import run_bass_kernel_spmd

F32 = mybir.dt.float32
BF16 = mybir.dt.bfloat16
AF = mybir.ActivationFunctionType
ALU = mybir.AluOpType

# problem dims (hardcoded per contract)
T, HID, H = 2048, 5120, 16
QL, KL = 1536, 512
NOPE, ROPE, VD = 128, 64, 128
QK = NOPE + ROPE
EPS = 1e-6
NCORE = 8
HPC = H // NCORE          # heads per core = 2
TLOC = T // NCORE         # tokens per core = 256
P = 128
HCH = HID // P            # 40 hidden chunks
QLC = QL // P             # 12
KLC = KL // P             # 4
MT = QLC + KLC + 1        # 17 a-proj output tiles (12 q + 4 kv + 1 pe[64])
NKV = MT - QLC            # 5 kv-group tiles
NT = T // P               # 16 token tiles
NQS = 4                   # 512-wide q slices per head
NHS = HID // 512          # 10 output column slices

# yarn rope params
BASE, FACTOR = 10000.0, 40.0
BETA_FAST, BETA_SLOW, ORIG_MAX = 32.0, 1.0, 4096
MSCALE = 1.0
MSCALE_ALL_DIM = 1.0


def _yarn_get_mscale(scale, m):
    if scale <= 1.0:
        return 1.0
    return 0.1 * m * math.log(scale) + 1.0


def _yarn_inv_freq():
    pos_freqs = BASE ** (np.arange(0, ROPE, 2, dtype=np.float64) / ROPE)
    extra = 1.0 / pos_freqs
    inter = 1.0 / (FACTOR * pos_freqs)

    def corr_dim(n):
        return ROPE * math.log(ORIG_MAX / (n * 2 * math.pi)) / (2 * math.log(BASE))

    low = max(math.floor(corr_dim(BETA_FAST)), 0)
    high = min(math.ceil(corr_dim(BETA_SLOW)), ROPE - 1)
    ramp = np.clip(
        (np.arange(ROPE // 2, dtype=np.float64) - low) / max(high - low, 0.001),
        0.0,
        1.0,
    )
    mask = 1.0 - ramp
    return (inter * (1.0 - mask) + extra * mask).astype(np.float32)


COS_SIN_MSCALE = _yarn_get_mscale(FACTOR, MSCALE) / _yarn_get_mscale(
    FACTOR, MSCALE_ALL_DIM
)
_M = _yarn_get_mscale(FACTOR, MSCALE_ALL_DIM)
ATTN_SCALE = (QK ** -0.5) * _M * _M

BF = ml_dtypes.bfloat16
# de-interleave perm: even rope dims then odd rope dims
PE_PERM = np.concatenate([np.arange(0, ROPE, 2), np.arange(1, ROPE, 2)])

LAST_EXEC_NS = None


def _build_nc(single=False, reps=1):
    # single=True: no collective, 1 core — for cost-model timeline sims only
    nc = bacc.Bacc(
        "TRN2",
        target_bir_lowering=False,
        debug=False,
        num_devices=1 if single else NCORE,
    )

    hT = nc.dram_tensor("hT", [P, HCH, TLOC], BF16, kind="ExternalInput").ap()
    wa = nc.dram_tensor("wa", [MT, P, HCH, P], BF16, kind="ExternalInput").ap()
    wqb = nc.dram_tensor("wqb", [P, QLC, HPC * QK], BF16, kind="ExternalInput").ap()
    wkvb = nc.dram_tensor("wkvb", [P, KLC, 512], BF16, kind="ExternalInput").ap()
    wo = nc.dram_tensor("wo", [P, HPC, HID], BF16, kind="ExternalInput").ap()
    cosf = nc.dram_tensor("cosf", [ROPE // 2, T], BF16, kind="ExternalInput").ap()
    sinf = nc.dram_tensor("sinf", [ROPE // 2, T], BF16, kind="ExternalInput").ap()
    cosl = nc.dram_tensor("cosl", [ROPE, TLOC], BF16, kind="ExternalInput").ap()
    sinl = nc.dram_tensor("sinl", [ROPE, TLOC], BF16, kind="ExternalInput").ap()
    onesd = nc.dram_tensor("ones", [P, P], BF16, kind="ExternalInput").ap()
    trid = nc.dram_tensor("tri", [P, P], BF16, kind="ExternalInput").ap()
    out = nc.dram_tensor("out", [T, HID], BF16, kind="ExternalOutput").ap()

    locb = nc.dram_tensor("locb", [MT, P, TLOC], BF16).ap()
    locr = nc.dram_tensor("locr", [P, TLOC], F32).ap()
    gathkv = nc.dram_tensor(
        "gathkv", [NCORE, NKV, P, TLOC], BF16, addr_space="Shared"
    ).ap()
    # q latents gathered RAW (pre-norm) in 2-mtile chunks as the a-proj
    # produces them; the rsq row-norms gather separately and are applied
    # after the q b-projection (scaling commutes through the matmul)
    gathq = [
        nc.dram_tensor(f"gathq{i}", [NCORE, 2, P, TLOC], BF16, addr_space="Shared").ap()
        for i in range(QLC // 2)
    ]
    gathr = nc.dram_tensor("gathr", [NCORE, P, TLOC], F32, addr_space="Shared").ap()

    with tile.TileContext(nc) as tc:
        with (
            tc.tile_pool(name="const", bufs=1) as cp,
            tc.tile_pool(name="persist", bufs=1) as pp,
            tc.tile_pool(name="ocp", bufs=3) as ocp,
        ):
            ones_sb = cp.tile([P, P], BF16, tag="ones")
            tri_sb = cp.tile([P, P], BF16, tag="tri")
            cosf_sb = cp.tile([ROPE // 2, T], BF16, tag="cosf")
            sinf_sb = cp.tile([ROPE // 2, T], BF16, tag="sinf")
            cosl_sb = cp.tile([ROPE, TLOC], BF16, tag="cosl")
            sinl_sb = cp.tile([ROPE, TLOC], BF16, tag="sinl")
            eps_sb = cp.tile([P, 1], F32, tag="eps")
            nc.vector.memset(eps_sb[:], EPS)

            # persistent attention operands (live across the phase transition)
            qTn = pp.tile([P, HPC, T], BF16, tag="qTn")
            # both heads' roped q_pe packed: rows [h0e h0o h1e h1o] x 32
            qTp = pp.tile([P, T], BF16, tag="qTp")
            kTn = pp.tile([P, HPC, T], BF16, tag="kTn")
            vtok = pp.tile([P, NT, HPC * VD], BF16, tag="vtok")
            OnT = pp.tile([P, HPC, T], BF16, tag="OnT")
            # k_pe duplicated into both 64-row halves so each head's score
            # matmul has lhsT/rhs at the same base partition (0 or 64)
            kpe = pp.tile([P, NCORE, TLOC], BF16, tag="kpe")
            wkvb_sb = pp.tile([P, KLC, 512], BF16, tag="wkvb")
            wqb_sb = pp.tile([P, QLC, HPC * QK], BF16, tag="wqb")
            # raw q latents (all cores) + per-token rsq, filled during phase 1
            qag = pp.tile([P, QLC, NCORE, TLOC], BF16, tag="qag")
            qag_f = qag.rearrange("p m c t -> p m (c t)")
            rsqf = pp.tile([P, NCORE, TLOC], F32, tag="rsqf")
            rsqf_f = rsqf.rearrange("p c t -> p (c t)")

            for _rep in range(reps):
                # ---------------- phase 1: local a-projections ----------------
                with (
                    tc.tile_pool(name="p1", bufs=1) as p1,
                    tc.tile_pool(name="wap", bufs=2) as wap,
                    tc.tile_pool(name="sqp", bufs=3) as sqp,
                    tc.tile_pool(name="ps1", bufs=3, space="PSUM") as ps1,
                    tc.tile_pool(name="pss", bufs=1, space="PSUM") as pss,
                ):
                    hT_sb = p1.tile([P, HCH, TLOC], BF16, tag="hT")
                    # only the first tiny chunk up front; the rest is
                    # interleaved with the first weight tile's chunks below
                    nc.sync.dma_start(hT_sb[:, 0:2, :], hT[:, 0:2, :])
                    araw = p1.tile([P, MT, TLOC], BF16, tag="araw")
                    anrm = p1.tile([P, NKV, TLOC], BF16, tag="anrm")
                    # kv gather buffer lives only in phase 1 (kv_b consumes it)
                    kag = p1.tile([P, KLC, NCORE, TLOC], BF16, tag="kag")
                    kag_f = kag.rearrange("p m c t -> p m (c t)")
                    ssq = pss.tile([P, TLOC], F32, tag="ssq")
                    sskv = pss.tile([P, TLOC], F32, tag="sskv")

                    pend_sq = None

                    def emit_ssq(sq, sm):
                        if sm < QLC:
                            nc.tensor.matmul(
                                ssq[:], ones_sb[:], sq[:],
                                start=(sm == 0), stop=(sm == QLC - 1),
                                skip_group_check=True,
                            )
                        else:
                            nc.tensor.matmul(
                                sskv[:], ones_sb[:], sq[:],
                                start=(sm == QLC), stop=(sm == QLC + KLC - 1),
                                skip_group_check=True,
                            )

                    # kv-group mtiles first so their collective + the whole kv
                    # b-projection overlap the (3x bigger) q-group a-proj
                    for m in list(range(QLC, MT)) + list(range(QLC)):
                        wt = wap.tile([P, HCH, P], BF16, tag="wt")
                        if m == QLC:  # first mtile: pace-matched interleave of
                            # weight chunks and the rest of hT so the k-loop
                            # never waits long on either stream
                            nc.sync.dma_start(wt[:, 0:5, :], wa[m, :, 0:5, :])
                            nc.sync.dma_start(ones_sb[:], onesd)
                            nc.sync.dma_start(cosl_sb[:], cosl)
                            nc.sync.dma_start(sinl_sb[:], sinl)
                            for (w0, w1), (h0, h1) in [
                                ((5, 14), (2, 8)),
                                ((14, 24), (8, 16)),
                                ((24, 40), (16, 28)),
                                ((40, 40), (28, 40)),
                            ]:
                                if w1 > w0:
                                    nc.sync.dma_start(
                                        wt[:, w0:w1, :], wa[m, :, w0:w1, :]
                                    )
                                nc.sync.dma_start(hT_sb[:, h0:h1, :], hT[:, h0:h1, :])
                        else:
                            nc.sync.dma_start(wt[:], wa[m])
                            if m == QLC - 2:
                                # phase-2 q weights: late enough to not crowd
                                # the wa stream, early enough for phase 2
                                nc.sync.dma_start(wqb_sb[:], wqb)
                        ps = ps1.tile([P, TLOC], F32, tag="aps")
                        for k in range(HCH):
                            nc.tensor.matmul(
                                ps[:],
                                wt[:, k, :],
                                hT_sb[:, k, :],
                                start=(k == 0),
                                stop=(k == HCH - 1),
                            )
                        nc.scalar.copy(araw[:, m, :], ps[:])
                        # sum-of-squares matmuls run one mtile late so the
                        # Act copy+Square chain hides under the next a-proj
                        if pend_sq is not None:
                            emit_ssq(*pend_sq)
                            pend_sq = None
                        if m < QLC + KLC:
                            sq = sqp.tile([P, TLOC], BF16, tag="sq")
                            nc.scalar.activation(sq[:], ps[:], AF.Square)
                            pend_sq = (sq, m)

                        if m < QLC and m % 2 == 1:
                            if m == QLC - 1:
                                # final ssq + rsq chain first: the tiny rsq
                                # gather must beat the last chunk's gathers
                                # into the DMA queues (phase 2 needs it first)
                                emit_ssq(*pend_sq)
                                pend_sq = None
                                rsq_q = p1.tile([P, TLOC], F32, tag="rsq_q")
                                tmpf = p1.tile([P, TLOC], F32, tag="tmpf")
                                nc.scalar.activation(
                                    tmpf[:], ssq[:], AF.Sqrt,
                                    bias=eps_sb[:], scale=1.0 / QL,
                                )
                                nc.vector.reciprocal(rsq_q[:], tmpf[:])
                                nc.sync.dma_start(locr, rsq_q[:])
                                if not single:
                                    nc.gpsimd.collective_compute(
                                        "AllGather",
                                        ALU.bypass,
                                        replica_groups=[list(range(NCORE))],
                                        ins=[locr.opt()],
                                        outs=[gathr.opt()],
                                    )
                                nc.sync.dma_start(
                                    rsqf[:], gathr.rearrange("c p t -> p c t")
                                )
                            # ship this RAW q-latent pair: write, gather, and
                            # pull into SBUF while later mtiles still compute
                            cch = m // 2
                            nc.sync.dma_start(
                                locb[m - 1 : m + 1].rearrange("m p t -> p m t"),
                                araw[:, m - 1 : m + 1, :],
                            )
                            if not single:
                                nc.gpsimd.collective_compute(
                                    "AllGather",
                                    ALU.bypass,
                                    replica_groups=[list(range(NCORE))],
                                    ins=[locb[m - 1 : m + 1].opt()],
                                    outs=[gathq[cch].opt()],
                                )
                            for j in range(2):
                                nc.sync.dma_start(
                                    qag[:, m - 1 + j],
                                    gathq[cch][:, j].rearrange("c p t -> p c t"),
                                )

                        if m == MT - 1:
                            # kv group locally complete: normalize, rope, ship
                            rsq_k = p1.tile([P, TLOC], F32, tag="rsq_k")
                            tmpf2 = p1.tile([P, TLOC], F32, tag="tmpf2")
                            nc.scalar.activation(
                                tmpf2[:], sskv[:], AF.Sqrt,
                                bias=eps_sb[:], scale=1.0 / KL,
                            )
                            nc.vector.reciprocal(rsq_k[:], tmpf2[:])
                            for mm in range(QLC, QLC + KLC):
                                nc.vector.tensor_mul(
                                    anrm[:, mm - QLC, :], araw[:, mm, :], rsq_k[:]
                                )
                            # rope k_pe (rows 0:32 even, 32:64 odd of tile MT-1).
                            # Two-SBUF-input ops must share base partition, so
                            # cos/sin tables are duplicated across both halves.
                            t1 = p1.tile([ROPE, TLOC], BF16, tag="t1")
                            t2 = p1.tile([ROPE, TLOC], BF16, tag="t2")
                            xe = araw[0:32, MT - 1, :]
                            xo = araw[32:64, MT - 1, :]
                            nc.vector.tensor_mul(t1[0:32, :], xe, cosl_sb[0:32, :])
                            nc.vector.tensor_mul(t2[0:32, :], xo, sinl_sb[32:64, :])
                            nc.vector.tensor_sub(
                                anrm[0:32, NKV - 1, :], t1[0:32, :], t2[0:32, :]
                            )
                            nc.vector.tensor_mul(t1[32:64, :], xo, cosl_sb[32:64, :])
                            nc.vector.tensor_mul(t2[32:64, :], xe, sinl_sb[0:32, :])
                            nc.vector.tensor_add(
                                anrm[32:64, NKV - 1, :], t1[32:64, :], t2[32:64, :]
                            )
                            nc.vector.memset(anrm[64:128, NKV - 1, :], 0.0)
                            nc.sync.dma_start(
                                locb[QLC:MT].rearrange("m p t -> p m t"),
                                anrm[:, 0:NKV, :],
                            )
                            if not single:
                                nc.gpsimd.collective_compute(
                                    "AllGather",
                                    ALU.bypass,
                                    replica_groups=[list(range(NCORE))],
                                    ins=[locb[QLC:MT].opt()],
                                    outs=[gathkv.opt()],
                                )
                            # kv gather-in + whole kv b-projection — overlaps
                            # the q-group a-proj matmuls still streaming on PE
                            nc.sync.dma_start(wkvb_sb[:], wkvb)
                            for mm in range(KLC):
                                nc.sync.dma_start(
                                    kag[:, mm],
                                    gathkv[:, mm].rearrange("c p t -> p c t"),
                                )
                            for half in range(2):
                                nc.sync.dma_start(
                                    kpe[half * ROPE : (half + 1) * ROPE],
                                    gathkv[:, NKV - 1, 0:ROPE, :].rearrange(
                                        "c p t -> p c t"
                                    ),
                                )
                            # k_nope^T per head: [128, T]
                            for hh in range(HPC):
                                for s in range(4):
                                    psb = ps1.tile([P, 512], F32, tag="bp")
                                    for k in range(KLC):
                                        nc.tensor.matmul(
                                            psb[:],
                                            wkvb_sb[:, k, hh * 128 : (hh + 1) * 128],
                                            kag_f[:, k, s * 512 : (s + 1) * 512],
                                            start=(k == 0),
                                            stop=(k == KLC - 1),
                                        )
                                    nc.scalar.copy(
                                        kTn[:, hh, s * 512 : (s + 1) * 512], psb[:]
                                    )
                            # v token-major: [t, 2*VD] per token tile
                            for tt in range(NT):
                                psb = ps1.tile([P, 512], F32, tag="bp")
                                for k in range(KLC):
                                    nc.tensor.matmul(
                                        psb[:, 0 : HPC * VD],
                                        kag[
                                            :, k, tt // 2,
                                            (tt % 2) * 128 : (tt % 2) * 128 + 128,
                                        ],
                                        wkvb_sb[:, k, 256:512],
                                        start=(k == 0),
                                        stop=(k == KLC - 1),
                                    )
                                nc.vector.tensor_copy(
                                    vtok[:, tt, :], psb[:, 0 : HPC * VD]
                                )

                    if pend_sq is not None:
                        emit_ssq(*pend_sq)
                        pend_sq = None

                # ---------------- phase 2: q b-proj + attention + w_o ----------
                with (
                    tc.tile_pool(name="p2", bufs=1) as p2,
                    tc.tile_pool(name="ptp", bufs=2) as ptp,
                    tc.tile_pool(name="rcp", bufs=2) as rcp,
                    tc.tile_pool(name="dsp", bufs=2) as dsp,
                    tc.tile_pool(name="psB", bufs=3, space="PSUM") as psB,
                    tc.tile_pool(name="psA", bufs=3, space="PSUM") as psA,
                ):
                    # w_o weights only live in phase 2
                    wo_sb = p2.tile([P, HPC, HID], BF16, tag="wo")
                    nc.sync.dma_start(cosf_sb[:], cosf)
                    nc.sync.dma_start(sinf_sb[:], sinf)
                    nc.sync.dma_start(tri_sb[:], trid)
                    nc.sync.dma_start(wo_sb[:], wo)

                    # q^T: nope [128, T] per head; both heads' pe packed M=128
                    # (wqb cols: [h0 nope | h1 nope | h0 pe | h1 pe]);
                    # rsq row-norm folded into the PSUM->SBUF move
                    def nope_slice(s):
                        for hh in range(HPC):
                            ps = psB.tile([P, 512], F32, tag="bp")
                            for k in range(QLC):
                                nc.tensor.matmul(
                                    ps[:],
                                    wqb_sb[:, k, hh * NOPE : (hh + 1) * NOPE],
                                    qag_f[:, k, s * 512 : (s + 1) * 512],
                                    start=(k == 0),
                                    stop=(k == QLC - 1),
                                )
                            nc.vector.tensor_mul(
                                qTn[:, hh, s * 512 : (s + 1) * 512],
                                ps[:],
                                rsqf_f[:, s * 512 : (s + 1) * 512],
                            )

                    def q_pe_slice(s):
                        ps = psB.tile([P, 512], F32, tag="bp")
                        for k in range(QLC):
                            nc.tensor.matmul(
                                ps[:],
                                wqb_sb[:, k, HPC * NOPE : HPC * QK],
                                qag_f[:, k, s * 512 : (s + 1) * 512],
                                start=(k == 0),
                                stop=(k == QLC - 1),
                            )
                        # rope both heads' pe straight out of PSUM
                        # (PSUM x SBUF ops are exempt from the equal-base rule)
                        sl = slice(s * 512, (s + 1) * 512)
                        cs, sn = cosf_sb[:, sl], sinf_sb[:, sl]
                        rt = p2.tile([P, 512], BF16, tag="rt")
                        for hh in range(HPC):
                            b = hh * ROPE
                            xe, xo = ps[b : b + 32, :], ps[b + 32 : b + 64, :]
                            nc.vector.tensor_mul(qTp[b : b + 32, sl], xe, cs)
                            nc.vector.tensor_mul(rt[b : b + 32, :], xo, sn)
                            nc.vector.tensor_sub(
                                qTp[b : b + 32, sl],
                                qTp[b : b + 32, sl],
                                rt[b : b + 32, :],
                            )
                            nc.vector.tensor_mul(qTp[b + 32 : b + 64, sl], xo, cs)
                            nc.vector.tensor_mul(rt[b + 32 : b + 64, :], xe, sn)
                            nc.vector.tensor_add(
                                qTp[b + 32 : b + 64, sl],
                                qTp[b + 32 : b + 64, sl],
                                rt[b + 32 : b + 64, :],
                            )
                        # apply the q rsq row-norm (commutes with rope)
                        nc.vector.tensor_mul(
                            qTp[:, sl], qTp[:, sl], rsqf_f[:, sl]
                        )

                    # w_o as a queue of 512-col chunk-group emitters so the
                    # attention kt loops can drain them as PE filler (keeps PE
                    # ahead of the Act exp stream)
                    def make_wo_queue(qs):
                        items = []
                        for tt in range(4 * qs, 4 * qs + 4):
                            for half in range(2):
                                for hc in range(NHS // 2):
                                    items.append((tt, half, hc))
                        ctx = {"i": 0}

                        def emit_one():
                            if ctx["i"] >= len(items):
                                return False
                            tt, half, hc = items[ctx["i"]]
                            ctx["i"] += 1
                            if hc == 0:
                                oc_new = ocp.tile([P, HID // 2], BF16, tag="oc")
                                ctx["oc"] = oc_new
                            oc = ctx["oc"]
                            hs = half * (NHS // 2) + hc
                            ps_f = psB.tile([P, 512], F32, tag="bp")
                            nc.tensor.matmul(
                                ps_f[:],
                                OnT[:, 0, tt * 128 : (tt + 1) * 128],
                                wo_sb[:, 0, hs * 512 : (hs + 1) * 512],
                                start=True,
                                stop=False,
                            )
                            nc.tensor.matmul(
                                ps_f[:],
                                OnT[:, 1, tt * 128 : (tt + 1) * 128],
                                wo_sb[:, 1, hs * 512 : (hs + 1) * 512],
                                start=False,
                                stop=True,
                            )
                            if hc % 2 == 0:
                                nc.scalar.copy(
                                    oc[:, hc * 512 : (hc + 1) * 512], ps_f[:]
                                )
                            else:
                                nc.vector.tensor_copy(
                                    oc[:, hc * 512 : (hc + 1) * 512], ps_f[:]
                                )
                            if tt == NT - 1:
                                # finest tail: per-chunk DMA so the drain only
                                # waits on the last 512 columns
                                nc.sync.dma_start(
                                    out[
                                        tt * 128 : (tt + 1) * 128,
                                        hs * 512 : (hs + 1) * 512,
                                    ],
                                    oc[:, hc * 512 : (hc + 1) * 512],
                                )
                            elif hc == NHS // 2 - 1:
                                nc.sync.dma_start(
                                    out[
                                        tt * 128 : (tt + 1) * 128,
                                        half * (HID // 2) : (half + 1) * (HID // 2),
                                    ],
                                    oc[:],
                                )
                            return True

                        return emit_one

                    def wo_flush(emit_one):
                        if emit_one is not None:
                            while emit_one():
                                pass

                    # attention (S^T layout, no max-subtraction); softmax
                    # denominator accumulated on DVE + Pool partition-reduce
                    # instead of PE ones-matmuls
                    def attn(qs, filler=None, nfill=0):
                        for hh in range(HPC):
                            nk = 4 * qs + 4
                            PT = ptp.tile([P, NT, 512], BF16, tag="PT")
                            # two partial accumulators: even k-tiles on DVE,
                            # odd on Pool — splits the add load across engines
                            dsum = dsp.tile([P, 512], F32, tag="dsum")
                            dsumB = dsp.tile([P, 512], F32, tag="dsumB")
                            for kt in range(nk):
                                # columns 0..r*128 are fully causal-masked:
                                # skip them in every matmul of this k-tile
                                r = kt - 4 * qs
                                c0 = max(r, 0) * 128
                                ps_s = psA.tile([P, 512], F32, tag="ps_s")
                                nc.tensor.matmul(
                                    ps_s[:, c0:512],
                                    kTn[:, hh, kt * 128 : (kt + 1) * 128],
                                    qTn[:, hh, qs * 512 + c0 : (qs + 1) * 512],
                                    start=True,
                                    stop=False,
                                )
                                hb = hh * ROPE
                                nc.tensor.matmul(
                                    ps_s[:, c0:512],
                                    kpe[
                                        hb : hb + ROPE, kt // 2,
                                        (kt % 2) * 128 : (kt % 2) * 128 + 128,
                                    ],
                                    qTp[hb : hb + ROPE, qs * 512 + c0 : (qs + 1) * 512],
                                    start=False,
                                    stop=True,
                                )
                                if c0 > 0:
                                    nc.vector.memset(PT[:, kt, 0:c0], 0.0)
                                nc.scalar.activation(
                                    PT[:, kt, c0:512], ps_s[:, c0:512], AF.Exp
                                )
                                if 0 <= r <= 3:
                                    nc.vector.tensor_mul(
                                        PT[:, kt, r * 128 : (r + 1) * 128],
                                        PT[:, kt, r * 128 : (r + 1) * 128],
                                        tri_sb[:],
                                    )
                                eng = nc.vector if kt % 2 == 0 else nc.gpsimd
                                acc = dsum if kt % 2 == 0 else dsumB
                                if kt < 2:
                                    eng.tensor_copy(acc[:], PT[:, kt, :])
                                else:
                                    eng.tensor_add(acc[:], acc[:], PT[:, kt, :])
                                if filler is not None and (kt > 0 or hh > 0):
                                    for _ in range(nfill):
                                        filler()
                            ps_o = psB.tile([P, 512], F32, tag="bp")
                            for kt in range(nk):
                                c0 = max(kt - 4 * qs, 0) * 128
                                nc.tensor.matmul(
                                    ps_o[:, c0:512],
                                    vtok[:, kt, hh * VD : (hh + 1) * VD],
                                    PT[:, kt, c0:512],
                                    start=(kt == 0),
                                    stop=(kt == nk - 1),
                                )
                            dred = dsp.tile([P, 512], F32, tag="dred")
                            nc.vector.tensor_add(dsum[:], dsum[:], dsumB[:])
                            nc.gpsimd.partition_all_reduce(
                                dred[:], dsum[:], channels=128,
                                reduce_op=bass_isa.ReduceOp.add,
                            )
                            rec = rcp.tile([P, 512], F32, tag="rec")
                            nc.vector.reciprocal(rec[:], dred[:])
                            nc.vector.tensor_mul(
                                OnT[:, hh, qs * 512 : (qs + 1) * 512], ps_o[:], rec[:]
                            )

                    # staggered schedule: attention trails the q b-proj by one
                    # slice; w_o of slice qs drains inside attention qs+1's
                    # kt loops as PE filler (PE stays ahead of the exp stream)
                    nope_slice(0)
                    q_pe_slice(0)
                    nope_slice(1)
                    q_pe_slice(1)
                    attn(0)
                    nope_slice(2)
                    q_pe_slice(2)
                    wq0 = make_wo_queue(0)
                    attn(1, filler=wq0, nfill=3)
                    wo_flush(wq0)
                    nope_slice(3)
                    q_pe_slice(3)
                    wq1 = make_wo_queue(1)
                    attn(2, filler=wq1, nfill=2)
                    wo_flush(wq1)
                    wq2 = make_wo_queue(2)
                    attn(3, filler=wq2, nfill=1)
                    wo_flush(wq2)
                    wq3 = make_wo_queue(3)
                    wo_flush(wq3)

    nc.compile()
    return nc


_NC_CACHE = None


def _get_nc():
    global _NC_CACHE
    if _NC_CACHE is None:
        _NC_CACHE = _build_nc()
    return _NC_CACHE


def _bf(x):
    return np.ascontiguousarray(x.astype(BF))


def _prep_in_maps(inputs):
    hidden = np.asarray(inputs["hidden_states"], dtype=np.float32)
    w_q_a = np.asarray(inputs["w_q_a"], dtype=np.float32)
    q_a_norm_w = np.asarray(inputs["q_a_norm_w"], dtype=np.float32)
    w_q_b = np.asarray(inputs["w_q_b"], dtype=np.float32)
    w_kv_a = np.asarray(inputs["w_kv_a"], dtype=np.float32)
    kv_a_norm_w = np.asarray(inputs["kv_a_norm_w"], dtype=np.float32)
    w_kv_b = np.asarray(inputs["w_kv_b"], dtype=np.float32)
    w_o = np.asarray(inputs["w_o"], dtype=np.float32)
    pos = np.asarray(inputs["positions"]).astype(np.float32)

    # rope tables, feature-major, evens/odds share the same row index
    inv_freq = _yarn_inv_freq()
    freqs = pos[:, None] * inv_freq[None, :]          # [T, 32]
    cosf = np.cos(freqs).T * COS_SIN_MSCALE           # [32, T]
    sinf = np.sin(freqs).T * COS_SIN_MSCALE
    cosf_b, sinf_b = _bf(cosf), _bf(sinf)
    cosl2 = np.concatenate([cosf_b, cosf_b], 0)       # duplicated halves
    sinl2 = np.concatenate([sinf_b, sinf_b], 0)

    # a-proj weights: [17 mtiles, 128p, 40k, 128c], pe cols de-interleaved
    wkva_pe = w_kv_a[:, KL:][:, PE_PERM]
    wa_full = np.concatenate(
        [w_q_a, w_kv_a[:, :KL], wkva_pe, np.zeros((HID, 64), np.float32)], axis=1
    )  # [5120, 2176]
    wa_l = _bf(wa_full.reshape(HCH, P, MT, P).transpose(2, 1, 0, 3))

    # fold RMSNorm gains + attention scale into b-proj weights
    wqb_s = w_q_b * q_a_norm_w[:, None] * ATTN_SCALE
    wkvb_s = w_kv_b * kv_a_norm_w[:, None]

    ones_b = _bf(np.ones((P, P), np.float32))
    tri_b = _bf(np.triu(np.ones((P, P), np.float32)))

    shared = {
        "wa": wa_l,
        "cosf": cosf_b,
        "sinf": sinf_b,
        "ones": ones_b,
        "tri": tri_b,
    }

    in_maps = []
    for c in range(NCORE):
        h0 = HPC * c
        # hidden slice, feature-major [128, 40, 256]
        hs = hidden[c * TLOC : (c + 1) * TLOC, :]
        hT_l = _bf(hs.T.reshape(HCH, P, TLOC).transpose(1, 0, 2))
        # w_q_b cols for this core's heads: [h0 nope | h1 nope | h0 pe | h1 pe]
        nope_cols, pe_cols = [], []
        for h in range(h0, h0 + HPC):
            blk = wqb_s[:, h * QK : (h + 1) * QK]
            nope_cols.append(blk[:, :NOPE])
            pe_cols.append(blk[:, NOPE:][:, PE_PERM])
        wqb_core = np.concatenate(nope_cols + pe_cols, axis=1)  # [1536, 384]
        wqb_l = _bf(wqb_core.reshape(QLC, P, HPC * QK).transpose(1, 0, 2))
        # w_kv_b cols: [h0 nope, h1 nope, h0 v, h1 v]
        nopes = [
            wkvb_s[:, h * (NOPE + VD) : h * (NOPE + VD) + NOPE]
            for h in range(h0, h0 + HPC)
        ]
        vs = [
            wkvb_s[:, h * (NOPE + VD) + NOPE : (h + 1) * (NOPE + VD)]
            for h in range(h0, h0 + HPC)
        ]
        wkvb_core = np.concatenate(nopes + vs, axis=1)  # [512, 512]
        wkvb_l = _bf(wkvb_core.reshape(KLC, P, 512).transpose(1, 0, 2))
        # w_o rows for this core's heads: [128, 2, 5120]
        wo_core = w_o[h0 * VD : (h0 + HPC) * VD, :]
        wo_l = _bf(wo_core.reshape(HPC, P, HID).transpose(1, 0, 2))

        m = dict(shared)
        m.update(
            {
                "hT": hT_l,
                "wqb": wqb_l,
                "wkvb": wkvb_l,
                "wo": wo_l,
                "cosl": np.ascontiguousarray(cosl2[:, c * TLOC : (c + 1) * TLOC]),
                "sinl": np.ascontiguousarray(sinl2[:, c * TLOC : (c + 1) * TLOC]),
            }
        )
        in_maps.append(m)
    return in_maps


def kernel(**inputs):
    global LAST_EXEC_NS
    nc = _get_nc()
    in_maps = _prep_in_maps(inputs)
    trace = os.environ.get("KERNEL_TRACE", "0") == "1"
    res = run_bass_kernel_spmd(
        nc, in_maps, core_ids=list(range(NCORE)), trace=trace
    )
    LAST_EXEC_NS = res.exec_time_ns
    out = res.results[0]["out"].astype(np.float32)
    for i in range(1, NCORE):
        out += res.results[i]["out"].astype(np.float32)
    return out

